# revision 48
# baseline (speedup 1.0000x reference)
"""CapsNet forward kernel for Trainium2, 8-core data-parallel.

Strategy (per spec sharding_hint): batch (512) split across 8 cores (64 each);
all params replicated. Routing logits b are a batch-mean -> AllGather of
per-core partial deltas (1152 floats) per routing round (rounds 1,2 only;
round 3's b update is dead in the reference).

Math restructuring (keeps exact semantics, avoids materializing u):
  r := s*1152 + n  (s=caps idx, n=(c32,oy,ox))  == co*36 + pix  with co=s*32+c32
  xr2[b, r]   = primary-caps output (relu), flattened
  W2n[r, hl]  = W.transpose(3,0,1,2).reshape(9216,160)
  s[b,hl]  = sum_r c[n(r)] * W2n[r,hl] * xr2[b,r]        (matmul, K=9216)
  v        = squash_dim1(s)
  G[r,hl]  = sum_b xr2[b,r] * v[b,hl]                    (matmul, K=64/core)
  delta[n] = 1/(B*160) * sum_s sum_hl W2n[r,hl]*G[r,hl]  (DVE TT-reduce)
Convs are PE matmuls: conv1 via in-SBUF "wide patch" im2col (K=81),
primary-caps conv via 81 shifted-window matmuls accumulated in PSUM (K=256).
All big matmuls run as float32r (full-rate fp32 PE mode).
"""

import numpy as np

import concourse.bass as bass
import concourse.mybir as mybir
import concourse.tile as tile
from concourse.ap import AP
from concourse.bass_utils import run_bass_kernel_spmd

F32 = mybir.dt.float32
F32R = mybir.dt.float32r
AL = mybir.AluOpType
AF = mybir.ActivationFunctionType
AX = mybir.AxisListType

NCORES = 8
B = 512
BC = B // NCORES           # 64 images per core
MAX_WAITS = 1              # walrus on this path allows 1 sync wait per inst
HL = 160                   # 10 classes x 16 pose
NS = 9216                  # 1152 caps x 8
NT = NS // 128             # 72 K-tiles
GROUPS = [(0, 14), (14, 14), (28, 14), (42, 14), (56, 8)]  # conv2 image groups
ROUTE_SCALE = 1.0 / (B * HL)


def _r(t, dims):
    """Raw AP on tile/ap t with explicit [step, count] dims (elements)."""
    return AP(t.tensor, t.offset, dims)


def split_waits(nc, max_waits=MAX_WAITS):
    """This walrus build rejects >max_waits sync waits per instruction; move
    excess waits onto same-engine NoOps inserted immediately before."""
    for f in nc.m.functions:
        for blk in f.blocks:
            out = []
            for ins in blk.instructions:
                si = ins.sync_info
                if si is not None and si.on_wait and len(si.on_wait) > max_waits:
                    waits = list(si.on_wait)
                    k = 0
                    while len(waits) > max_waits:
                        chunk, waits = waits[:max_waits], waits[max_waits:]
                        nop = mybir.InstNoOp(name=f"{ins.name}-ws{k}", ins=[], outs=[])
                        nop.engine = ins.engine
                        nop.sync_info = mybir.SyncInfo(on_wait=chunk, on_update=[])
                        out.append(nop)
                        k += 1
                    ins.sync_info = mybir.SyncInfo(
                        on_wait=waits, on_update=list(si.on_update or []))
                out.append(ins)
            blk.instructions = out


def build_nc(stub_collective=False):
    nc = bass.Bass(num_devices=1 if stub_collective else NCORES)

    xs = nc.dram_tensor("xs", [BC, 800], F32R, kind="ExternalInput")
    w1t = nc.dram_tensor("w1t", [81, 256], F32R, kind="ExternalInput")
    b1 = nc.dram_tensor("b1", [256], F32, kind="ExternalInput")
    pcwt = nc.dram_tensor("pcwt", [81, 256, 256], F32R, kind="ExternalInput")
    pcb = nc.dram_tensor("pcb", [256], F32, kind="ExternalInput")
    w2n = nc.dram_tensor("w2n", [NS, HL], F32R, kind="ExternalInput")
    w2nt = nc.dram_tensor("w2nt", [HL, NS], F32R, kind="ExternalInput")
    eye64 = nc.dram_tensor("eye64", [BC, BC], F32R, kind="ExternalInput")
    vout = nc.dram_tensor("vout", [BC, HL], F32R, kind="ExternalOutput")

    pc_rd = nc.dram_tensor("pc_rd", [NS, BC], F32R)    # [r, b]

    with tile.TileContext(nc) as tc:
        with (
            tc.tile_pool(name="pers", bufs=1) as pers,
            tc.tile_pool(name="dram", bufs=1, space="DRAM") as dpool,
        ):
            w1t_sb = pers.tile([81, 256], F32R)
            nc.sync.dma_start(w1t_sb[:], w1t[:])
            b1_sb = pers.tile([128, 2], F32)
            nc.sync.dma_start(b1_sb[:], _r(b1[:], [[1, 128], [128, 2]]))
            pcb_sb = pers.tile([128, 2], F32)
            nc.sync.dma_start(pcb_sb[:], _r(pcb[:], [[1, 128], [128, 2]]))
            ones128 = pers.tile([128, 1], F32)
            nc.gpsimd.memset(ones128[:], 1.0)
            ones1 = pers.tile([1, 128], F32)
            nc.gpsimd.memset(ones1[:], 1.0)
            b9 = pers.tile([128, 9], F32)
            eye_sb = pers.tile([BC, BC], F32R)
            nc.sync.dma_start(eye_sb[:], eye64[:])

            # ---------------- conv phase ----------------
            with (
                tc.tile_pool(name="convsb", bufs=1) as csb,
                tc.tile_pool(name="pwp", bufs=3) as pwp,
                tc.tile_pool(name="ps1p", bufs=2, space="PSUM") as ps1p,
                tc.tile_pool(name="ps2p", bufs=2, space="PSUM") as ps2p,
            ):
                acc0 = csb.tile([128, BC * 36], F32)
                acc1 = csb.tile([128, BC * 36], F32)
                accs = [acc0, acc1]
                for ci_blk in range(2):
                    h1 = csb.tile([128, BC * 400], F32R, tag="h1")
                    hp = h1.ap[0][0]
                    for i in range(BC):
                        pw = pwp.tile([81, 560], F32R, tag="pw")
                        nc.sync.dma_start(
                            pw[:],
                            AP(xs[:].tensor, i * 800, [[28, 9], [1, 9], [1, 560]]),
                        )
                        ps1 = ps1p.tile([128, 400], F32, tag="ps1")
                        rhs = _r(pw, [[pw.ap[0][0], 81], [28, 20], [1, 20]])
                        out4 = _r(ps1, [[ps1.ap[0][0], 128], [20, 20], [1, 20]])
                        nc.tensor.matmul(
                            out4,
                            w1t_sb[:, ci_blk * 128:(ci_blk + 1) * 128],
                            rhs,
                            start=True, stop=True,
                        )
                        nc.scalar.activation(
                            h1[:, i * 400:(i + 1) * 400], ps1[:], AF.Relu,
                            bias=b1_sb[:, ci_blk:ci_blk + 1],
                        )
                    for co_blk in range(2):
                        w2c = csb.tile([128, 81 * 128], F32R, tag="w2c")
                        nc.sync.dma_start(
                            w2c[:],
                            AP(pcwt[:].tensor,
                               ci_blk * 128 * 256 + co_blk * 128,
                               [[256, 128], [256 * 256, 81], [1, 128]]),
                        )
                        for (g0, nb) in GROUPS:
                            ps2 = ps2p.tile([128, 504], F32, tag="ps2")
                            pstep = ps2.ap[0][0]
                            for kk in range(81):
                                ky, kx = divmod(kk, 9)
                                rhs = AP(h1.tensor,
                                         h1.offset + g0 * 400 + ky * 20 + kx,
                                         [[hp, 128], [400, nb], [40, 6], [2, 6]])
                                out4 = _r(ps2, [[pstep, 128], [36, nb], [6, 6], [1, 6]])
                                nc.tensor.matmul(
                                    out4,
                                    w2c[:, kk * 128:(kk + 1) * 128],
                                    rhs,
                                    start=(kk == 0), stop=(kk == 80),
                                )
                            dst = accs[co_blk][:, g0 * 36:(g0 + nb) * 36]
                            if ci_blk == 0:
                                nc.scalar.copy(dst, ps2[:, :nb * 36])
                            else:
                                nc.vector.tensor_tensor(dst, dst, ps2[:, :nb * 36], AL.add)
                # bias + relu -> pc2 (pix-major) -> pc_rd[r, b] in DRAM
                for co_blk in range(2):
                    pc2 = csb.tile([128, BC * 36], F32R, tag="pc2")
                    p2 = pc2.ap[0][0]
                    nc.scalar.activation(
                        _r(pc2, [[p2, 128], [1, BC], [BC, 36]]),
                        _r(accs[co_blk], [[accs[co_blk].ap[0][0], 128], [36, BC], [1, 36]]),
                        AF.Relu,
                        bias=pcb_sb[:, co_blk:co_blk + 1],
                    )
                    nc.sync.dma_start(
                        AP(pc_rd[:].tensor, co_blk * 128 * 36 * BC,
                           [[36 * BC, 128], [BC, 36], [1, BC]]),
                        _r(pc2, [[p2, 128], [BC, 36], [1, BC]]),
                    )

            # ---------------- routing phase ----------------
            with (
                tc.tile_pool(name="rsb", bufs=1) as rsb,
                tc.tile_pool(name="rnd", bufs=2) as rnd,
                tc.tile_pool(name="sps", bufs=1, space="PSUM") as sps,
                tc.tile_pool(name="gps", bufs=4, space="PSUM") as gps,
                tc.tile_pool(name="zps", bufs=1, space="PSUM") as zps,
            ):
                w2sb = rsb.tile([128, NT * HL], F32R)
                nc.sync.dma_start(
                    w2sb[:],
                    AP(w2n[:].tensor, 0, [[HL, 128], [128 * HL, NT], [1, HL]]),
                )
                # W2n^T in two hl-chunks: (128, NT*128) + (32, NT*128)
                w2nt_a = rsb.tile([128, NT * 128], F32R)
                nc.sync.dma_start(
                    w2nt_a[:],
                    AP(w2nt[:].tensor, 0, [[NS, 128], [128, NT], [1, 128]]),
                )
                w2nt_b = rsb.tile([32, NT * 128], F32R)
                nc.sync.dma_start(
                    w2nt_b[:],
                    AP(w2nt[:].tensor, 128 * NS, [[NS, 32], [128, NT], [1, 128]]),
                )
                xrT = rsb.tile([128, NT * BC], F32R)
                nc.sync.dma_start(
                    xrT[:],
                    AP(pc_rd[:].tensor, 0, [[BC, 128], [128 * BC, NT], [1, BC]]),
                )
                p_all = rsb.tile([128, NT * BC], F32)
                prod = rsb.tile([128, (NT // 2) * BC], F32)

                def s_matmul():
                    s_ps = sps.tile([BC, HL], F32, tag="s_ps")
                    for t in range(NT):
                        nc.tensor.matmul(
                            s_ps[:],
                            xrT[:, t * BC:(t + 1) * BC],
                            w2sb[:, t * HL:(t + 1) * HL],
                            start=(t == 0), stop=(t == NT - 1),
                        )
                    return s_ps

                def squash(s_sb):
                    sq = rnd.tile([BC, HL], F32, tag="sq")
                    nc.scalar.square(sq[:], s_sb[:])
                    n2 = rnd.tile([BC, 16], F32, tag="n2")
                    nc.vector.tensor_reduce(
                        n2[:].rearrange("a b -> a b ()"),
                        _r(sq, [[sq.ap[0][0], BC], [1, 16], [16, 10]]),
                        AX.X, AL.add,
                    )
                    rt = rnd.tile([BC, 16], F32, tag="rt")
                    nc.scalar.sqrt(rt[:], n2[:])
                    n2p1 = rnd.tile([BC, 16], F32, tag="n2p1")
                    nc.vector.tensor_scalar_add(n2p1[:], n2[:], 1.0)
                    rcp = rnd.tile([BC, 16], F32, tag="rcp")
                    nc.vector.reciprocal(rcp[:], n2p1[:])
                    f = rnd.tile([BC, 16], F32, tag="f")
                    nc.vector.tensor_tensor(f[:], rt[:], rcp[:], AL.mult)
                    v_sb = rnd.tile([BC, HL], F32R, tag="v_sb")
                    nc.vector.tensor_tensor(
                        _r(v_sb, [[v_sb.ap[0][0], BC], [16, 10], [1, 16]]),
                        _r(s_sb, [[s_sb.ap[0][0], BC], [16, 10], [1, 16]]),
                        _r(f, [[f.ap[0][0], BC], [0, 10], [1, 16]]),
                        AL.mult,
                    )
                    return v_sb

                def p_delta_update(v_sb, rnd_idx, rce9):
                    """delta via P[r,b] = sum_hl W2n[r,hl] v[b,hl] (PE), then
                    D[r] = sum_b xrT[r,b]*P[r,b] (DVE). If xrT is c-scaled,
                    divide delta9 by ce9 (rce9 ap) to undo."""
                    vt_ps = gps.tile([128, BC], F32R, tag="vt_ps", bufs=1)
                    nc.tensor.transpose(vt_ps[:], v_sb[:, 0:128], eye_sb[:])
                    vt_a = rnd.tile([128, BC], F32R, tag="vt_a")
                    nc.scalar.copy(vt_a[:], vt_ps[:])
                    vtb_ps = gps.tile([32, BC], F32R, tag="vtb_ps", bufs=1)
                    nc.tensor.transpose(vtb_ps[:], v_sb[:, 128:160], eye_sb[:])
                    vt_b = rnd.tile([32, BC], F32R, tag="vt_b")
                    nc.scalar.copy(vt_b[:], vtb_ps[:])
                    for t in range(NT):
                        p_ps = gps.tile([128, BC], F32, tag="p_ps", bufs=3)
                        nc.tensor.matmul(
                            p_ps[:],
                            w2nt_a[:, t * 128:(t + 1) * 128],
                            vt_a[:],
                            start=True, stop=False,
                        )
                        nc.tensor.matmul(
                            p_ps[:],
                            w2nt_b[:, t * 128:(t + 1) * 128],
                            vt_b[:],
                            start=False, stop=True,
                        )
                        nc.scalar.copy(p_all[:, t * BC:(t + 1) * BC], p_ps[:])
                    D = rnd.tile([128, NT], F32, tag="D")
                    half = (NT // 2) * BC
                    for hx in range(2):
                        nc.vector.tensor_tensor(
                            prod[:],
                            xrT[:, hx * half:(hx + 1) * half].bitcast(F32),
                            p_all[:, hx * half:(hx + 1) * half],
                            AL.mult,
                        )
                        nc.vector.tensor_reduce(
                            D[:, hx * (NT // 2):(hx + 1) * (NT // 2)]
                            .rearrange("a b -> a b ()"),
                            _r(prod, [[prod.ap[0][0], 128], [BC, NT // 2], [1, BC]]),
                            AX.X, AL.add,
                        )
                    delta9 = rnd.tile([128, 9], F32, tag="delta9")
                    nc.vector.tensor_reduce(
                        delta9[:].rearrange("a b -> a b ()"),
                        _r(D, [[D.ap[0][0], 128], [1, 9], [9, 8]]),
                        AX.X, AL.add,
                    )
                    if rce9 is not None:
                        nc.vector.tensor_tensor(delta9[:], delta9[:], rce9[:], AL.mult)
                    cin = dpool.tile([128, 9], F32, name=f"cin{rnd_idx}")
                    cout = dpool.tile([NCORES * 128, 9], F32, name=f"cout{rnd_idx}",
                                      addr_space=("Local" if stub_collective else "Shared"))
                    nc.gpsimd.dma_start(cin[:], delta9[:])
                    if stub_collective:
                        for cc in range(NCORES):
                            nc.gpsimd.dma_start(
                                AP(cout.tensor, cout.offset + cc * 1152,
                                   [[9, 128], [1, 9]]),
                                delta9[:],
                            )
                    else:
                        nc.gpsimd.collective_compute(
                            "AllGather", AL.bypass,
                            replica_groups=[list(range(NCORES))],
                            ins=[cin.opt()], outs=[cout.opt()],
                        )
                    agg = rnd.tile([128, 8 * 9], F32, tag="agg")
                    nc.gpsimd.dma_start(
                        agg[:],
                        AP(cout.tensor, cout.offset, [[9, 128], [1, 9], [128 * 9, 8]]),
                    )
                    dsum = rnd.tile([128, 9], F32, tag="dsum")
                    nc.vector.tensor_reduce(
                        dsum[:].rearrange("a b -> a b ()"),
                        _r(agg, [[agg.ap[0][0], 128], [1, 9], [9, 8]]),
                        AX.X, AL.add,
                    )
                    if rnd_idx == 0:
                        nc.scalar.mul(b9[:], dsum[:], ROUTE_SCALE)
                    else:
                        sc = rnd.tile([128, 9], F32, tag="sc")
                        nc.scalar.mul(sc[:], dsum[:], ROUTE_SCALE)
                        nc.vector.tensor_tensor(b9[:], b9[:], sc[:], AL.add)

                def softmax_ce9():
                    """ce9[p,j] = softmax(b9)[n=j*128+p], F32R (128,9)."""
                    e9 = rnd.tile([128, 9], F32, tag="e9")
                    nc.scalar.activation(e9[:], b9[:], AF.Exp)
                    rs9 = rnd.tile([128, 1], F32, tag="rs9")
                    nc.vector.tensor_reduce(
                        rs9[:].rearrange("a b -> a b ()"), e9[:], AX.X, AL.add)
                    z_ps = zps.tile([1, 1], F32, tag="z_ps")
                    nc.tensor.matmul(z_ps[:], ones128[:], rs9[:], start=True, stop=True)
                    z_sb = rnd.tile([1, 1], F32, tag="z_sb")
                    nc.scalar.copy(z_sb[:], z_ps[:])
                    zb_ps = zps.tile([128, 1], F32, tag="zb_ps")
                    nc.tensor.matmul(zb_ps[:], ones1[:], z_sb[:], start=True, stop=True)
                    rz = rnd.tile([128, 1], F32, tag="rz")
                    nc.vector.reciprocal(rz[:], zb_ps[:])
                    ce9 = rnd.tile([128, 9], F32R, tag="ce9")
                    nc.vector.tensor_scalar_mul(ce9[:], e9[:], rz[:])
                    return ce9

                def scale_xrT(m9):
                    """xrT[p, (q,j,b)] *= m9[p, j] in place."""
                    nc.vector.tensor_tensor(
                        _r(xrT, [[xrT.ap[0][0], 128], [9 * BC, 8], [BC, 9], [1, BC]]),
                        _r(xrT, [[xrT.ap[0][0], 128], [9 * BC, 8], [BC, 9], [1, BC]]),
                        _r(m9, [[m9.ap[0][0], 128], [0, 8], [1, 9], [0, BC]]),
                        AL.mult,
                    )

                # ---- round 1 (c uniform; xrT unscaled) ----
                s_ps = s_matmul()
                s_sb = rnd.tile([BC, HL], F32, tag="s_sb")
                nc.scalar.mul(s_sb[:], s_ps[:], 1.0 / 1152.0)
                v_sb = squash(s_sb)
                p_delta_update(v_sb, 0, None)
                # ---- round 2 ----
                ce9_2 = softmax_ce9()
                scale_xrT(ce9_2)
                rce9 = rnd.tile([128, 9], F32, tag="rce9")
                nc.vector.reciprocal(rce9[:], ce9_2[:].bitcast(F32))
                s_ps = s_matmul()
                s_sb = rnd.tile([BC, HL], F32, tag="s_sb")
                nc.scalar.copy(s_sb[:], s_ps[:])
                v_sb = squash(s_sb)
                p_delta_update(v_sb, 1, rce9)
                # ---- round 3 (b update dead) ----
                ce9_3 = softmax_ce9()
                ratio9 = rnd.tile([128, 9], F32R, tag="ratio9")
                nc.vector.tensor_tensor(ratio9[:], ce9_3[:].bitcast(F32), rce9[:], AL.mult)
                scale_xrT(ratio9)
                s_ps = s_matmul()
                s_sb = rnd.tile([BC, HL], F32, tag="s_sb")
                nc.scalar.copy(s_sb[:], s_ps[:])
                v_sb = squash(s_sb)
                nc.sync.dma_start(vout[:], v_sb[:])

    return nc


_NC_CACHE = None


def _get_nc():
    global _NC_CACHE
    if _NC_CACHE is None:
        nc = build_nc()
        split_waits(nc)
        _NC_CACHE = nc
    return _NC_CACHE


def prepare_inputs(x, conv1_w, conv1_b, pc_w, pc_b, W):
    x = np.asarray(x, np.float32)
    xs = np.zeros((B, 800), np.float32)
    xs[:, :784] = x.reshape(B, 784)
    w1t = np.ascontiguousarray(np.asarray(conv1_w, np.float32).reshape(256, 81).T)
    b1 = np.ascontiguousarray(np.asarray(conv1_b, np.float32))
    pcwt = np.ascontiguousarray(
        np.asarray(pc_w, np.float32).reshape(256, 256, 81).transpose(2, 1, 0))
    pcb = np.ascontiguousarray(np.asarray(pc_b, np.float32).reshape(256))
    w2n = np.ascontiguousarray(
        np.asarray(W, np.float32).transpose(3, 0, 1, 2).reshape(NS, HL))
    w2nt = np.ascontiguousarray(w2n.T)
    eye64 = np.eye(BC, dtype=np.float32)
    in_maps = []
    for c in range(NCORES):
        in_maps.append({
            "xs": np.ascontiguousarray(xs[c * BC:(c + 1) * BC]),
            "w1t": w1t, "b1": b1, "pcwt": pcwt, "pcb": pcb, "w2n": w2n,
            "w2nt": w2nt, "eye64": eye64,
        })
    return in_maps


def kernel(x, conv1_w, conv1_b, pc_w, pc_b, W, _trace=False, _trace_kwargs=None):
    nc = _get_nc()
    in_maps = prepare_inputs(x, conv1_w, conv1_b, pc_w, pc_b, W)
    res = run_bass_kernel_spmd(
        nc, in_maps, list(range(NCORES)),
        trace=_trace, **(_trace_kwargs or {}),
    )
    v = np.concatenate([np.asarray(res.results[c]["vout"]) for c in range(NCORES)], 0)
    out = v.reshape(B, 1, 1, 10, 16).astype(np.float32)
    if _trace:
        return out, res
    return out


# revision 56
# speedup vs baseline: 1.1790x; 1.1790x over previous
"""CapsNet forward kernel for Trainium2, 8-core data-parallel.

Strategy (per spec sharding_hint): batch (512) split across 8 cores (64 each);
all params replicated. Routing logits b are a batch-mean -> AllGather of
per-core partial deltas (1152 floats) per routing round (rounds 1,2 only;
round 3's b update is dead in the reference).

Math restructuring (keeps exact semantics, avoids materializing u):
  r := s*1152 + n  (s=caps idx, n=(c32,oy,ox))  == co*36 + pix  with co=s*32+c32
  xr2[b, r]   = primary-caps output (relu), flattened
  W2n[r, hl]  = W.transpose(3,0,1,2).reshape(9216,160)
  s[b,hl]  = sum_r c[n(r)] * W2n[r,hl] * xr2[b,r]        (matmul, K=9216)
  v        = squash_dim1(s)
  G[r,hl]  = sum_b xr2[b,r] * v[b,hl]                    (matmul, K=64/core)
  delta[n] = 1/(B*160) * sum_s sum_hl W2n[r,hl]*G[r,hl]  (DVE TT-reduce)
Convs are PE matmuls: conv1 via in-SBUF "wide patch" im2col (K=81),
primary-caps conv via 81 shifted-window matmuls accumulated in PSUM (K=256).
All big matmuls run as float32r (full-rate fp32 PE mode).
"""

import numpy as np

import concourse.bass as bass
import concourse.mybir as mybir
import concourse.tile as tile
from concourse.ap import AP
from concourse.bass_utils import run_bass_kernel_spmd

F32 = mybir.dt.float32
F32R = mybir.dt.float32r
BF16 = mybir.dt.bfloat16
NPBF16 = mybir.dt.np(mybir.dt.bfloat16)
AL = mybir.AluOpType
AF = mybir.ActivationFunctionType
AX = mybir.AxisListType

NCORES = 8
B = 512
BC = B // NCORES           # 64 images per core
MAX_WAITS = 1              # walrus on this path allows 1 sync wait per inst
HL = 160                   # 10 classes x 16 pose
NS = 9216                  # 1152 caps x 8
NT = NS // 128             # 72 K-tiles
GROUPS = [(0, 14), (14, 14), (28, 14), (42, 14), (56, 8)]  # conv2 image groups
ROUTE_SCALE = 1.0 / (B * HL)


def _r(t, dims):
    """Raw AP on tile/ap t with explicit [step, count] dims (elements)."""
    return AP(t.tensor, t.offset, dims)


def split_waits(nc, max_waits=MAX_WAITS):
    """This walrus build rejects >max_waits sync waits per instruction; move
    excess waits onto same-engine NoOps inserted immediately before."""
    for f in nc.m.functions:
        for blk in f.blocks:
            out = []
            for ins in blk.instructions:
                si = ins.sync_info
                if si is not None and si.on_wait and len(si.on_wait) > max_waits:
                    waits = list(si.on_wait)
                    k = 0
                    while len(waits) > max_waits:
                        chunk, waits = waits[:max_waits], waits[max_waits:]
                        nop = mybir.InstNoOp(name=f"{ins.name}-ws{k}", ins=[], outs=[])
                        nop.engine = ins.engine
                        nop.sync_info = mybir.SyncInfo(on_wait=chunk, on_update=[])
                        out.append(nop)
                        k += 1
                    ins.sync_info = mybir.SyncInfo(
                        on_wait=waits, on_update=list(si.on_update or []))
                out.append(ins)
            blk.instructions = out


def build_nc(stub_collective=False):
    nc = bass.Bass(num_devices=1 if stub_collective else NCORES)

    xs = nc.dram_tensor("xs", [BC, 800], F32R, kind="ExternalInput")
    w1t = nc.dram_tensor("w1t", [81, 256], F32R, kind="ExternalInput")
    b1 = nc.dram_tensor("b1", [256], F32, kind="ExternalInput")
    pcwt = nc.dram_tensor("pcwt", [81, 256, 256], F32R, kind="ExternalInput")
    pcb = nc.dram_tensor("pcb", [256], F32, kind="ExternalInput")
    w2sb_h = nc.dram_tensor("w2sb_h", [128, NT * HL], BF16, kind="ExternalInput")
    w2nta_h = nc.dram_tensor("w2nta_h", [128, NS], BF16, kind="ExternalInput")
    w2ntb_h = nc.dram_tensor("w2ntb_h", [32, NS], BF16, kind="ExternalInput")
    eye64 = nc.dram_tensor("eye64", [BC, BC], F32R, kind="ExternalInput")
    vout = nc.dram_tensor("vout", [BC, HL], F32R, kind="ExternalOutput")

    pc_rd = nc.dram_tensor("pc_rd", [NS, BC], BF16)    # [r, b]

    with tile.TileContext(nc) as tc:
        with (
            tc.tile_pool(name="pers", bufs=1) as pers,
            tc.tile_pool(name="dram", bufs=1, space="DRAM") as dpool,
        ):
            w1t_sb = pers.tile([81, 256], F32R)
            nc.sync.dma_start(w1t_sb[:], w1t[:])
            b1_sb = pers.tile([128, 2], F32)
            nc.sync.dma_start(b1_sb[:], _r(b1[:], [[1, 128], [128, 2]]))
            pcb_sb = pers.tile([128, 2], F32)
            nc.sync.dma_start(pcb_sb[:], _r(pcb[:], [[1, 128], [128, 2]]))
            ones128 = pers.tile([128, 1], F32)
            nc.gpsimd.memset(ones128[:], 1.0)
            ones1 = pers.tile([1, 128], F32)
            nc.gpsimd.memset(ones1[:], 1.0)
            b9 = pers.tile([128, 9], F32)
            eye_sb = pers.tile([BC, BC], F32R)
            nc.sync.dma_start(eye_sb[:], eye64[:])

            # ---------------- conv phase ----------------
            with (
                tc.tile_pool(name="convsb", bufs=1) as csb,
                tc.tile_pool(name="pwp", bufs=3) as pwp,
                tc.tile_pool(name="ps1p", bufs=2, space="PSUM") as ps1p,
                tc.tile_pool(name="ps2p", bufs=2, space="PSUM") as ps2p,
            ):
                acc0 = csb.tile([128, BC * 36], F32)
                acc1 = csb.tile([128, BC * 36], F32)
                accs = [acc0, acc1]
                for ci_blk in range(2):
                    h1 = csb.tile([128, BC * 400], F32R, tag="h1")
                    hp = h1.ap[0][0]
                    for i in range(BC):
                        pw = pwp.tile([81, 560], F32R, tag="pw")
                        nc.sync.dma_start(
                            pw[:],
                            AP(xs[:].tensor, i * 800, [[28, 9], [1, 9], [1, 560]]),
                        )
                        ps1 = ps1p.tile([128, 400], F32, tag="ps1")
                        rhs = _r(pw, [[pw.ap[0][0], 81], [28, 20], [1, 20]])
                        out4 = _r(ps1, [[ps1.ap[0][0], 128], [20, 20], [1, 20]])
                        nc.tensor.matmul(
                            out4,
                            w1t_sb[:, ci_blk * 128:(ci_blk + 1) * 128],
                            rhs,
                            start=True, stop=True,
                        )
                        nc.scalar.activation(
                            h1[:, i * 400:(i + 1) * 400], ps1[:], AF.Relu,
                            bias=b1_sb[:, ci_blk:ci_blk + 1],
                        )
                    for co_blk in range(2):
                        w2c = csb.tile([128, 81 * 128], F32R, tag="w2c")
                        nc.sync.dma_start(
                            w2c[:],
                            AP(pcwt[:].tensor,
                               ci_blk * 128 * 256 + co_blk * 128,
                               [[256, 128], [256 * 256, 81], [1, 128]]),
                        )
                        for (g0, nb) in GROUPS:
                            ps2 = ps2p.tile([128, 504], F32, tag="ps2")
                            pstep = ps2.ap[0][0]
                            for kk in range(81):
                                ky, kx = divmod(kk, 9)
                                rhs = AP(h1.tensor,
                                         h1.offset + g0 * 400 + ky * 20 + kx,
                                         [[hp, 128], [400, nb], [40, 6], [2, 6]])
                                out4 = _r(ps2, [[pstep, 128], [36, nb], [6, 6], [1, 6]])
                                nc.tensor.matmul(
                                    out4,
                                    w2c[:, kk * 128:(kk + 1) * 128],
                                    rhs,
                                    start=(kk == 0), stop=(kk == 80),
                                )
                            dst = accs[co_blk][:, g0 * 36:(g0 + nb) * 36]
                            if ci_blk == 0:
                                nc.scalar.copy(dst, ps2[:, :nb * 36])
                            else:
                                nc.vector.tensor_tensor(dst, dst, ps2[:, :nb * 36], AL.add)
                # bias + relu -> pc2 (pix-major, bf16) -> pc_rd[r, b] in DRAM
                for co_blk in range(2):
                    pc2 = csb.tile([128, BC * 36], BF16, tag="pc2")
                    p2 = pc2.ap[0][0]
                    nc.scalar.activation(
                        _r(pc2, [[p2, 128], [1, BC], [BC, 36]]),
                        _r(accs[co_blk], [[accs[co_blk].ap[0][0], 128], [36, BC], [1, 36]]),
                        AF.Relu,
                        bias=pcb_sb[:, co_blk:co_blk + 1],
                    )
                    nc.sync.dma_start(
                        AP(pc_rd[:].tensor, co_blk * 128 * 36 * BC,
                           [[36 * BC, 128], [BC, 36], [1, BC]]),
                        _r(pc2, [[p2, 128], [BC, 36], [1, BC]]),
                    )

            # ---------------- routing phase ----------------
            with (
                tc.tile_pool(name="rsb", bufs=1) as rsb,
                tc.tile_pool(name="rnd", bufs=2) as rnd,
                tc.tile_pool(name="sps", bufs=1, space="PSUM") as sps,
                tc.tile_pool(name="gps", bufs=4, space="PSUM") as gps,
                tc.tile_pool(name="zps", bufs=1, space="PSUM") as zps,
            ):
                # xrT first so s_matmul can start while weights stream in;
                # w2sb split in two tiles so the first half unblocks early
                xrT = rsb.tile([128, NT * BC], BF16)
                nc.sync.dma_start(
                    xrT[:],
                    AP(pc_rd[:].tensor, 0, [[BC, 128], [128 * BC, NT], [1, BC]]),
                )
                w2sb = [rsb.tile([128, 36 * HL], BF16, name=f"w2sb{h}")
                        for h in range(2)]
                for h in range(2):
                    nc.sync.dma_start(
                        w2sb[h][:],
                        AP(w2sb_h[:].tensor, h * 36 * HL,
                           [[NT * HL, 128], [1, 36 * HL]]),
                    )
                w2nt_a = rsb.tile([128, NT * 128], BF16)
                nc.sync.dma_start(w2nt_a[:], w2nta_h[:])
                w2nt_b = rsb.tile([32, NT * 128], BF16)
                nc.sync.dma_start(w2nt_b[:], w2ntb_h[:])
                p_all = rsb.tile([128, NT * BC], BF16)
                prod = rsb.tile([128, (NT // 2) * BC], F32)

                def s_matmul():
                    s_ps = sps.tile([BC, HL], F32, tag="s_ps")
                    for t in range(NT):
                        nc.tensor.matmul(
                            s_ps[:],
                            xrT[:, t * BC:(t + 1) * BC],
                            w2sb[t // 36][:, (t % 36) * HL:(t % 36 + 1) * HL],
                            start=(t == 0), stop=(t == NT - 1),
                        )
                    return s_ps

                def squash(s_sb):
                    sq = rnd.tile([BC, HL], F32, tag="sq")
                    nc.scalar.square(sq[:], s_sb[:])
                    n2 = rnd.tile([BC, 16], F32, tag="n2")
                    nc.vector.tensor_reduce(
                        n2[:].rearrange("a b -> a b ()"),
                        _r(sq, [[sq.ap[0][0], BC], [1, 16], [16, 10]]),
                        AX.X, AL.add,
                    )
                    rt = rnd.tile([BC, 16], F32, tag="rt")
                    nc.scalar.sqrt(rt[:], n2[:])
                    n2p1 = rnd.tile([BC, 16], F32, tag="n2p1")
                    nc.vector.tensor_scalar_add(n2p1[:], n2[:], 1.0)
                    rcp = rnd.tile([BC, 16], F32, tag="rcp")
                    nc.vector.reciprocal(rcp[:], n2p1[:])
                    f = rnd.tile([BC, 16], F32, tag="f")
                    nc.vector.tensor_tensor(f[:], rt[:], rcp[:], AL.mult)
                    v_sb = rnd.tile([BC, HL], F32R, tag="v_sb")
                    nc.vector.tensor_tensor(
                        _r(v_sb, [[v_sb.ap[0][0], BC], [16, 10], [1, 16]]),
                        _r(s_sb, [[s_sb.ap[0][0], BC], [16, 10], [1, 16]]),
                        _r(f, [[f.ap[0][0], BC], [0, 10], [1, 16]]),
                        AL.mult,
                    )
                    return v_sb

                def p_delta_update(v_sb, rnd_idx, rce9):
                    """delta via P[r,b] = sum_hl W2n[r,hl] v[b,hl] (PE), then
                    D[r] = sum_b xrT[r,b]*P[r,b] (DVE). If xrT is c-scaled,
                    divide delta9 by ce9 (rce9 ap) to undo."""
                    vt_ps = gps.tile([128, BC], F32R, tag="vt_ps", bufs=1)
                    nc.tensor.transpose(vt_ps[:], v_sb[:, 0:128], eye_sb[:])
                    vt_a = rnd.tile([128, BC], BF16, tag="vt_a")
                    nc.scalar.copy(vt_a[:], vt_ps[:])
                    vtb_ps = gps.tile([32, BC], F32R, tag="vtb_ps", bufs=1)
                    nc.tensor.transpose(vtb_ps[:], v_sb[:, 128:160], eye_sb[:])
                    vt_b = rnd.tile([32, BC], BF16, tag="vt_b")
                    nc.scalar.copy(vt_b[:], vtb_ps[:])
                    # 4 K-tiles per PSUM bank -> one psum->bf16 copy per 4
                    for g in range(NT // 4):
                        pps = gps.tile([128, 4 * BC], F32, tag="p_ps", bufs=3)
                        for q in range(4):
                            t = g * 4 + q
                            reg = pps[:, q * BC:(q + 1) * BC]
                            nc.tensor.matmul(
                                reg,
                                w2nt_a[:, t * 128:(t + 1) * 128],
                                vt_a[:],
                                start=True, stop=False,
                            )
                            nc.tensor.matmul(
                                reg,
                                w2nt_b[:, t * 128:(t + 1) * 128],
                                vt_b[:],
                                start=False, stop=True,
                            )
                        nc.scalar.copy(
                            p_all[:, g * 4 * BC:(g + 1) * 4 * BC], pps[:])
                    D = rnd.tile([128, NT], F32, tag="D")
                    half = (NT // 2) * BC
                    for hx in range(2):
                        nc.vector.tensor_tensor(
                            prod[:],
                            xrT[:, hx * half:(hx + 1) * half],
                            p_all[:, hx * half:(hx + 1) * half],
                            AL.mult,
                        )
                        nc.vector.tensor_reduce(
                            D[:, hx * (NT // 2):(hx + 1) * (NT // 2)]
                            .rearrange("a b -> a b ()"),
                            _r(prod, [[prod.ap[0][0], 128], [BC, NT // 2], [1, BC]]),
                            AX.X, AL.add,
                        )
                    delta9 = rnd.tile([128, 9], F32, tag="delta9")
                    nc.vector.tensor_reduce(
                        delta9[:].rearrange("a b -> a b ()"),
                        _r(D, [[D.ap[0][0], 128], [1, 9], [9, 8]]),
                        AX.X, AL.add,
                    )
                    if rce9 is not None:
                        nc.vector.tensor_tensor(delta9[:], delta9[:], rce9[:], AL.mult)
                    cin = dpool.tile([128, 9], F32, name=f"cin{rnd_idx}")
                    cout = dpool.tile([NCORES * 128, 9], F32, name=f"cout{rnd_idx}",
                                      addr_space=("Local" if stub_collective else "Shared"))
                    nc.gpsimd.dma_start(cin[:], delta9[:])
                    if stub_collective:
                        for cc in range(NCORES):
                            nc.gpsimd.dma_start(
                                AP(cout.tensor, cout.offset + cc * 1152,
                                   [[9, 128], [1, 9]]),
                                delta9[:],
                            )
                    else:
                        nc.gpsimd.collective_compute(
                            "AllGather", AL.bypass,
                            replica_groups=[list(range(NCORES))],
                            ins=[cin.opt()], outs=[cout.opt()],
                        )
                    agg = rnd.tile([128, 8 * 9], F32, tag="agg")
                    nc.gpsimd.dma_start(
                        agg[:],
                        AP(cout.tensor, cout.offset, [[9, 128], [1, 9], [128 * 9, 8]]),
                    )
                    dsum = rnd.tile([128, 9], F32, tag="dsum")
                    nc.vector.tensor_reduce(
                        dsum[:].rearrange("a b -> a b ()"),
                        _r(agg, [[agg.ap[0][0], 128], [1, 9], [9, 8]]),
                        AX.X, AL.add,
                    )
                    if rnd_idx == 0:
                        nc.scalar.mul(b9[:], dsum[:], ROUTE_SCALE)
                    else:
                        sc = rnd.tile([128, 9], F32, tag="sc")
                        nc.scalar.mul(sc[:], dsum[:], ROUTE_SCALE)
                        nc.vector.tensor_tensor(b9[:], b9[:], sc[:], AL.add)

                def softmax_ce9():
                    """ce9[p,j] = softmax(b9)[n=j*128+p]: (f32, bf16) pair."""
                    e9 = rnd.tile([128, 9], F32, tag="e9")
                    nc.scalar.activation(e9[:], b9[:], AF.Exp)
                    rs9 = rnd.tile([128, 1], F32, tag="rs9")
                    nc.vector.tensor_reduce(
                        rs9[:].rearrange("a b -> a b ()"), e9[:], AX.X, AL.add)
                    z_ps = zps.tile([1, 1], F32, tag="z_ps")
                    nc.tensor.matmul(z_ps[:], ones128[:], rs9[:], start=True, stop=True)
                    z_sb = rnd.tile([1, 1], F32, tag="z_sb")
                    nc.scalar.copy(z_sb[:], z_ps[:])
                    zb_ps = zps.tile([128, 1], F32, tag="zb_ps")
                    nc.tensor.matmul(zb_ps[:], ones1[:], z_sb[:], start=True, stop=True)
                    rz = rnd.tile([128, 1], F32, tag="rz")
                    nc.vector.reciprocal(rz[:], zb_ps[:])
                    ce9f = rnd.tile([128, 9], F32, tag="ce9f")
                    nc.vector.tensor_scalar_mul(ce9f[:], e9[:], rz[:])
                    ce9b = rnd.tile([128, 9], BF16, tag="ce9b")
                    nc.scalar.copy(ce9b[:], ce9f[:])
                    return ce9f, ce9b

                def scale_xrT(m9):
                    """xrT[p, (q,j,b)] *= m9[p, j] in place (bf16)."""
                    nc.vector.tensor_tensor(
                        _r(xrT, [[xrT.ap[0][0], 128], [9 * BC, 8], [BC, 9], [1, BC]]),
                        _r(xrT, [[xrT.ap[0][0], 128], [9 * BC, 8], [BC, 9], [1, BC]]),
                        _r(m9, [[m9.ap[0][0], 128], [0, 8], [1, 9], [0, BC]]),
                        AL.mult,
                    )

                # ---- round 1 (c uniform; xrT unscaled) ----
                s_ps = s_matmul()
                s_sb = rnd.tile([BC, HL], F32, tag="s_sb")
                nc.scalar.mul(s_sb[:], s_ps[:], 1.0 / 1152.0)
                v_sb = squash(s_sb)
                p_delta_update(v_sb, 0, None)
                # ---- round 2 ----
                ce9f_2, ce9b_2 = softmax_ce9()
                scale_xrT(ce9b_2)
                rce9 = rnd.tile([128, 9], F32, tag="rce9")
                nc.vector.reciprocal(rce9[:], ce9f_2[:])
                s_ps = s_matmul()
                s_sb = rnd.tile([BC, HL], F32, tag="s_sb")
                nc.scalar.copy(s_sb[:], s_ps[:])
                v_sb = squash(s_sb)
                p_delta_update(v_sb, 1, rce9)
                # ---- round 3 (b update dead) ----
                ce9f_3, _unused = softmax_ce9()
                ratio9f = rnd.tile([128, 9], F32, tag="ratio9f")
                nc.vector.tensor_tensor(ratio9f[:], ce9f_3[:], rce9[:], AL.mult)
                ratio9b = rnd.tile([128, 9], BF16, tag="ratio9b")
                nc.scalar.copy(ratio9b[:], ratio9f[:])
                scale_xrT(ratio9b)
                s_ps = s_matmul()
                s_sb = rnd.tile([BC, HL], F32, tag="s_sb")
                nc.scalar.copy(s_sb[:], s_ps[:])
                v_sb = squash(s_sb)
                nc.sync.dma_start(vout[:], v_sb[:])

    return nc


_NC_CACHE = None


def _get_nc():
    global _NC_CACHE
    if _NC_CACHE is None:
        nc = build_nc()
        split_waits(nc)
        _NC_CACHE = nc
    return _NC_CACHE


def prepare_inputs(x, conv1_w, conv1_b, pc_w, pc_b, W):
    x = np.asarray(x, np.float32)
    xs = np.zeros((B, 800), np.float32)
    xs[:, :784] = x.reshape(B, 784)
    w1t = np.ascontiguousarray(np.asarray(conv1_w, np.float32).reshape(256, 81).T)
    b1 = np.ascontiguousarray(np.asarray(conv1_b, np.float32))
    pcwt = np.ascontiguousarray(
        np.asarray(pc_w, np.float32).reshape(256, 256, 81).transpose(2, 1, 0))
    pcb = np.ascontiguousarray(np.asarray(pc_b, np.float32).reshape(256))
    w2n = np.ascontiguousarray(
        np.asarray(W, np.float32).transpose(3, 0, 1, 2).reshape(NS, HL))
    w2sb_h = np.ascontiguousarray(
        w2n.reshape(NT, 128, HL).transpose(1, 0, 2).reshape(128, NT * HL)
    ).astype(NPBF16)
    w2nt = np.ascontiguousarray(w2n.T)
    w2nta_h = np.ascontiguousarray(w2nt[:128]).astype(NPBF16)
    w2ntb_h = np.ascontiguousarray(w2nt[128:]).astype(NPBF16)
    eye64 = np.eye(BC, dtype=np.float32)
    in_maps = []
    for c in range(NCORES):
        in_maps.append({
            "xs": np.ascontiguousarray(xs[c * BC:(c + 1) * BC]),
            "w1t": w1t, "b1": b1, "pcwt": pcwt, "pcb": pcb,
            "w2sb_h": w2sb_h, "w2nta_h": w2nta_h, "w2ntb_h": w2ntb_h,
            "eye64": eye64,
        })
    return in_maps


def kernel(x, conv1_w, conv1_b, pc_w, pc_b, W, _trace=False, _trace_kwargs=None):
    nc = _get_nc()
    in_maps = prepare_inputs(x, conv1_w, conv1_b, pc_w, pc_b, W)
    res = run_bass_kernel_spmd(
        nc, in_maps, list(range(NCORES)),
        trace=_trace, **(_trace_kwargs or {}),
    )
    v = np.concatenate([np.asarray(res.results[c]["vout"]) for c in range(NCORES)], 0)
    out = v.reshape(B, 1, 1, 10, 16).astype(np.float32)
    if _trace:
        return out, res
    return out


# revision 58
# speedup vs baseline: 1.1941x; 1.0128x over previous
"""CapsNet forward kernel for Trainium2, 8-core data-parallel.

Strategy (per spec sharding_hint): batch (512) split across 8 cores (64 each);
all params replicated. Routing logits b are a batch-mean -> AllGather of
per-core partial deltas (1152 floats) per routing round (rounds 1,2 only;
round 3's b update is dead in the reference).

Math restructuring (keeps exact semantics, avoids materializing u):
  r := s*1152 + n  (s=caps idx, n=(c32,oy,ox))  == co*36 + pix  with co=s*32+c32
  xr2[b, r]   = primary-caps output (relu), flattened
  W2n[r, hl]  = W.transpose(3,0,1,2).reshape(9216,160)
  s[b,hl]  = sum_r c[n(r)] * W2n[r,hl] * xr2[b,r]        (matmul, K=9216)
  v        = squash_dim1(s)
  G[r,hl]  = sum_b xr2[b,r] * v[b,hl]                    (matmul, K=64/core)
  delta[n] = 1/(B*160) * sum_s sum_hl W2n[r,hl]*G[r,hl]  (DVE TT-reduce)
Convs are PE matmuls: conv1 via in-SBUF "wide patch" im2col (K=81),
primary-caps conv via 81 shifted-window matmuls accumulated in PSUM (K=256).
All big matmuls run as float32r (full-rate fp32 PE mode).
"""

import numpy as np

import concourse.bass as bass
import concourse.mybir as mybir
import concourse.tile as tile
from concourse.ap import AP
from concourse.bass_utils import run_bass_kernel_spmd

F32 = mybir.dt.float32
F32R = mybir.dt.float32r
BF16 = mybir.dt.bfloat16
NPBF16 = mybir.dt.np(mybir.dt.bfloat16)
AL = mybir.AluOpType
AF = mybir.ActivationFunctionType
AX = mybir.AxisListType

NCORES = 8
B = 512
BC = B // NCORES           # 64 images per core
MAX_WAITS = 1              # walrus on this path allows 1 sync wait per inst
HL = 160                   # 10 classes x 16 pose
NS = 9216                  # 1152 caps x 8
NT = NS // 128             # 72 K-tiles
GROUPS = [(0, 14), (14, 14), (28, 14), (42, 14), (56, 8)]  # conv2 image groups
ROUTE_SCALE = 1.0 / (B * HL)


def _r(t, dims):
    """Raw AP on tile/ap t with explicit [step, count] dims (elements)."""
    return AP(t.tensor, t.offset, dims)


def split_waits(nc, max_waits=MAX_WAITS):
    """This walrus build rejects >max_waits sync waits per instruction; move
    excess waits onto same-engine NoOps inserted immediately before."""
    for f in nc.m.functions:
        for blk in f.blocks:
            out = []
            for ins in blk.instructions:
                si = ins.sync_info
                if si is not None and si.on_wait and len(si.on_wait) > max_waits:
                    waits = list(si.on_wait)
                    k = 0
                    while len(waits) > max_waits:
                        chunk, waits = waits[:max_waits], waits[max_waits:]
                        nop = mybir.InstNoOp(name=f"{ins.name}-ws{k}", ins=[], outs=[])
                        nop.engine = ins.engine
                        nop.sync_info = mybir.SyncInfo(on_wait=chunk, on_update=[])
                        out.append(nop)
                        k += 1
                    ins.sync_info = mybir.SyncInfo(
                        on_wait=waits, on_update=list(si.on_update or []))
                out.append(ins)
            blk.instructions = out


def build_nc(stub_collective=False):
    nc = bass.Bass(num_devices=1 if stub_collective else NCORES)

    xs = nc.dram_tensor("xs", [BC, 800], F32R, kind="ExternalInput")
    w1t = nc.dram_tensor("w1t", [81, 256], F32R, kind="ExternalInput")
    b1 = nc.dram_tensor("b1", [256], F32, kind="ExternalInput")
    pcwt = nc.dram_tensor("pcwt", [81, 256, 256], F32R, kind="ExternalInput")
    pcb = nc.dram_tensor("pcb", [256], F32, kind="ExternalInput")
    w2sb_h = nc.dram_tensor("w2sb_h", [128, NT * HL], BF16, kind="ExternalInput")
    w2nta_h = nc.dram_tensor("w2nta_h", [128, NS], BF16, kind="ExternalInput")
    w2ntb_h = nc.dram_tensor("w2ntb_h", [32, NS], BF16, kind="ExternalInput")
    eye64 = nc.dram_tensor("eye64", [BC, BC], F32R, kind="ExternalInput")
    vout = nc.dram_tensor("vout", [BC, HL], F32R, kind="ExternalOutput")

    pc_rd = nc.dram_tensor("pc_rd", [NS, BC], BF16)    # [r, b]

    with tile.TileContext(nc) as tc:
        with (
            tc.tile_pool(name="pers", bufs=1) as pers,
            tc.tile_pool(name="dram", bufs=1, space="DRAM") as dpool,
        ):
            w1t_sb = pers.tile([81, 256], F32R)
            nc.sync.dma_start(w1t_sb[:], w1t[:])
            b1_sb = pers.tile([128, 2], F32)
            nc.sync.dma_start(b1_sb[:], _r(b1[:], [[1, 128], [128, 2]]))
            pcb_sb = pers.tile([128, 2], F32)
            nc.sync.dma_start(pcb_sb[:], _r(pcb[:], [[1, 128], [128, 2]]))
            ones128 = pers.tile([128, 1], F32)
            nc.gpsimd.memset(ones128[:], 1.0)
            ones1 = pers.tile([1, 128], F32)
            nc.gpsimd.memset(ones1[:], 1.0)
            b9 = pers.tile([128, 9], F32)
            eye_sb = pers.tile([BC, BC], F32R)
            nc.sync.dma_start(eye_sb[:], eye64[:])

            # ---------------- conv phase ----------------
            with (
                tc.tile_pool(name="convsb", bufs=1) as csb,
                tc.tile_pool(name="pwp", bufs=3) as pwp,
                tc.tile_pool(name="ps1p", bufs=2, space="PSUM") as ps1p,
                tc.tile_pool(name="ps2p", bufs=2, space="PSUM") as ps2p,
            ):
                acc0 = csb.tile([128, BC * 36], F32)
                acc1 = csb.tile([128, BC * 36], F32)
                accs = [acc0, acc1]
                for ci_blk in range(2):
                    h1 = csb.tile([128, BC * 400], F32R, tag="h1")
                    hp = h1.ap[0][0]
                    for i in range(BC):
                        pw = pwp.tile([81, 560], F32R, tag="pw")
                        nc.sync.dma_start(
                            pw[:],
                            AP(xs[:].tensor, i * 800, [[28, 9], [1, 9], [1, 560]]),
                        )
                        ps1 = ps1p.tile([128, 400], F32, tag="ps1")
                        rhs = _r(pw, [[pw.ap[0][0], 81], [28, 20], [1, 20]])
                        out4 = _r(ps1, [[ps1.ap[0][0], 128], [20, 20], [1, 20]])
                        nc.tensor.matmul(
                            out4,
                            w1t_sb[:, ci_blk * 128:(ci_blk + 1) * 128],
                            rhs,
                            start=True, stop=True,
                        )
                        nc.scalar.activation(
                            h1[:, i * 400:(i + 1) * 400], ps1[:], AF.Relu,
                            bias=b1_sb[:, ci_blk:ci_blk + 1],
                        )
                    for co_blk in range(2):
                        w2c = csb.tile([128, 81 * 128], F32R, tag="w2c")
                        nc.sync.dma_start(
                            w2c[:],
                            AP(pcwt[:].tensor,
                               ci_blk * 128 * 256 + co_blk * 128,
                               [[256, 128], [256 * 256, 81], [1, 128]]),
                        )
                        for (g0, nb) in GROUPS:
                            ps2 = ps2p.tile([128, 504], F32, tag="ps2")
                            pstep = ps2.ap[0][0]
                            for kk in range(81):
                                ky, kx = divmod(kk, 9)
                                rhs = AP(h1.tensor,
                                         h1.offset + g0 * 400 + ky * 20 + kx,
                                         [[hp, 128], [400, nb], [40, 6], [2, 6]])
                                out4 = _r(ps2, [[pstep, 128], [36, nb], [6, 6], [1, 6]])
                                nc.tensor.matmul(
                                    out4,
                                    w2c[:, kk * 128:(kk + 1) * 128],
                                    rhs,
                                    start=(kk == 0), stop=(kk == 80),
                                )
                            dst = accs[co_blk][:, g0 * 36:(g0 + nb) * 36]
                            if ci_blk == 0:
                                nc.scalar.copy(dst, ps2[:, :nb * 36])
                            else:
                                nc.vector.tensor_tensor(dst, dst, ps2[:, :nb * 36], AL.add)
                # bias + relu -> pc2 (pix-major, bf16) -> pc_rd[r, b] in DRAM
                for co_blk in range(2):
                    pc2 = csb.tile([128, BC * 36], BF16, tag="pc2")
                    p2 = pc2.ap[0][0]
                    nc.scalar.activation(
                        _r(pc2, [[p2, 128], [1, BC], [BC, 36]]),
                        _r(accs[co_blk], [[accs[co_blk].ap[0][0], 128], [36, BC], [1, 36]]),
                        AF.Relu,
                        bias=pcb_sb[:, co_blk:co_blk + 1],
                    )
                    nc.sync.dma_start(
                        AP(pc_rd[:].tensor, co_blk * 128 * 36 * BC,
                           [[36 * BC, 128], [BC, 36], [1, BC]]),
                        _r(pc2, [[p2, 128], [BC, 36], [1, BC]]),
                    )

            # ---------------- routing phase ----------------
            with (
                tc.tile_pool(name="rsb", bufs=1) as rsb,
                tc.tile_pool(name="rnd", bufs=2) as rnd,
                tc.tile_pool(name="sps", bufs=1, space="PSUM") as sps,
                tc.tile_pool(name="gps", bufs=4, space="PSUM") as gps,
                tc.tile_pool(name="zps", bufs=1, space="PSUM") as zps,
            ):
                # xrT first so s_matmul can start while weights stream in;
                # w2sb split in two tiles so the first half unblocks early
                xrT = rsb.tile([128, NT * BC], BF16)
                nc.sync.dma_start(
                    xrT[:],
                    AP(pc_rd[:].tensor, 0, [[BC, 128], [128 * BC, NT], [1, BC]]),
                )
                w2sb = [rsb.tile([128, 36 * HL], BF16, name=f"w2sb{h}")
                        for h in range(2)]
                for h in range(2):
                    nc.sync.dma_start(
                        w2sb[h][:],
                        AP(w2sb_h[:].tensor, h * 36 * HL,
                           [[NT * HL, 128], [1, 36 * HL]]),
                    )
                w2nt_a = rsb.tile([128, NT * 128], BF16)
                nc.sync.dma_start(w2nt_a[:], w2nta_h[:])
                w2nt_b = rsb.tile([32, NT * 128], BF16)
                nc.sync.dma_start(w2nt_b[:], w2ntb_h[:])
                p_all = rsb.tile([128, NT * BC], BF16)
                prod = rsb.tile([128, (NT // 2) * BC], F32)

                def s_matmul():
                    s_ps = sps.tile([BC, HL], F32, tag="s_ps")
                    for t in range(NT):
                        nc.tensor.matmul(
                            s_ps[:],
                            xrT[:, t * BC:(t + 1) * BC],
                            w2sb[t // 36][:, (t % 36) * HL:(t % 36 + 1) * HL],
                            start=(t == 0), stop=(t == NT - 1),
                        )
                    return s_ps

                def squash(s_sb):
                    sq = rnd.tile([BC, HL], F32, tag="sq")
                    nc.scalar.square(sq[:], s_sb[:])
                    n2 = rnd.tile([BC, 16], F32, tag="n2")
                    nc.vector.tensor_reduce(
                        n2[:].rearrange("a b -> a b ()"),
                        _r(sq, [[sq.ap[0][0], BC], [1, 16], [16, 10]]),
                        AX.X, AL.add,
                    )
                    rt = rnd.tile([BC, 16], F32, tag="rt")
                    nc.scalar.sqrt(rt[:], n2[:])
                    n2p1 = rnd.tile([BC, 16], F32, tag="n2p1")
                    nc.vector.tensor_scalar_add(n2p1[:], n2[:], 1.0)
                    rcp = rnd.tile([BC, 16], F32, tag="rcp")
                    nc.vector.reciprocal(rcp[:], n2p1[:])
                    f = rnd.tile([BC, 16], F32, tag="f")
                    nc.vector.tensor_tensor(f[:], rt[:], rcp[:], AL.mult)
                    v_sb = rnd.tile([BC, HL], F32R, tag="v_sb")
                    nc.vector.tensor_tensor(
                        _r(v_sb, [[v_sb.ap[0][0], BC], [16, 10], [1, 16]]),
                        _r(s_sb, [[s_sb.ap[0][0], BC], [16, 10], [1, 16]]),
                        _r(f, [[f.ap[0][0], BC], [0, 10], [1, 16]]),
                        AL.mult,
                    )
                    return v_sb

                def p_delta_update(v_sb, rnd_idx, rce9):
                    """delta via P[r,b] = sum_hl W2n[r,hl] v[b,hl] (PE), then
                    D[r] = sum_b xrT[r,b]*P[r,b] (DVE). If xrT is c-scaled,
                    divide delta9 by ce9 (rce9 ap) to undo."""
                    vt_ps = gps.tile([128, BC], F32R, tag="vt_ps", bufs=1)
                    nc.tensor.transpose(vt_ps[:], v_sb[:, 0:128], eye_sb[:])
                    vt_a = rnd.tile([128, BC], BF16, tag="vt_a")
                    nc.scalar.copy(vt_a[:], vt_ps[:])
                    vtb_ps = gps.tile([32, BC], F32R, tag="vtb_ps", bufs=1)
                    nc.tensor.transpose(vtb_ps[:], v_sb[:, 128:160], eye_sb[:])
                    vt_b = rnd.tile([32, BC], BF16, tag="vt_b")
                    nc.scalar.copy(vt_b[:], vtb_ps[:])
                    # 4 K-tiles per PSUM bank -> one psum->bf16 copy per 4
                    for g in range(NT // 4):
                        pps = gps.tile([128, 4 * BC], F32, tag="p_ps", bufs=3)
                        for q in range(4):
                            t = g * 4 + q
                            reg = pps[:, q * BC:(q + 1) * BC]
                            nc.tensor.matmul(
                                reg,
                                w2nt_a[:, t * 128:(t + 1) * 128],
                                vt_a[:],
                                start=True, stop=False,
                            )
                            nc.tensor.matmul(
                                reg,
                                w2nt_b[:, t * 128:(t + 1) * 128],
                                vt_b[:],
                                start=False, stop=True,
                            )
                        nc.scalar.copy(
                            p_all[:, g * 4 * BC:(g + 1) * 4 * BC], pps[:])
                    D = rnd.tile([128, NT], F32, tag="D")
                    half = (NT // 2) * BC
                    for hx in range(2):
                        nc.vector.tensor_tensor(
                            prod[:],
                            xrT[:, hx * half:(hx + 1) * half],
                            p_all[:, hx * half:(hx + 1) * half],
                            AL.mult,
                        )
                        nc.vector.tensor_reduce(
                            D[:, hx * (NT // 2):(hx + 1) * (NT // 2)]
                            .rearrange("a b -> a b ()"),
                            _r(prod, [[prod.ap[0][0], 128], [BC, NT // 2], [1, BC]]),
                            AX.X, AL.add,
                        )
                    delta9 = rnd.tile([128, 9], F32, tag="delta9")
                    nc.vector.tensor_reduce(
                        delta9[:].rearrange("a b -> a b ()"),
                        _r(D, [[D.ap[0][0], 128], [1, 9], [9, 8]]),
                        AX.X, AL.add,
                    )
                    if rce9 is not None:
                        nc.vector.tensor_tensor(delta9[:], delta9[:], rce9[:], AL.mult)
                    cin = dpool.tile([128, 9], F32, name=f"cin{rnd_idx}")
                    cout = dpool.tile([NCORES * 128, 9], F32, name=f"cout{rnd_idx}",
                                      addr_space=("Local" if stub_collective else "Shared"))
                    nc.sync.dma_start(cin[:], delta9[:])
                    if stub_collective:
                        for cc in range(NCORES):
                            nc.gpsimd.dma_start(
                                AP(cout.tensor, cout.offset + cc * 1152,
                                   [[9, 128], [1, 9]]),
                                delta9[:],
                            )
                    else:
                        nc.gpsimd.collective_compute(
                            "AllGather", AL.bypass,
                            replica_groups=[list(range(NCORES))],
                            ins=[cin.opt()], outs=[cout.opt()],
                        )
                    agg = rnd.tile([128, 8 * 9], F32, tag="agg")
                    nc.sync.dma_start(
                        agg[:],
                        AP(cout.tensor, cout.offset, [[9, 128], [1, 9], [128 * 9, 8]]),
                    )
                    dsum = rnd.tile([128, 9], F32, tag="dsum")
                    nc.vector.tensor_reduce(
                        dsum[:].rearrange("a b -> a b ()"),
                        _r(agg, [[agg.ap[0][0], 128], [1, 9], [9, 8]]),
                        AX.X, AL.add,
                    )
                    if rnd_idx == 0:
                        nc.scalar.mul(b9[:], dsum[:], ROUTE_SCALE)
                    else:
                        sc = rnd.tile([128, 9], F32, tag="sc")
                        nc.scalar.mul(sc[:], dsum[:], ROUTE_SCALE)
                        nc.vector.tensor_tensor(b9[:], b9[:], sc[:], AL.add)

                def softmax_ce9():
                    """ce9[p,j] = softmax(b9)[n=j*128+p]: (f32, bf16) pair."""
                    e9 = rnd.tile([128, 9], F32, tag="e9")
                    nc.scalar.activation(e9[:], b9[:], AF.Exp)
                    rs9 = rnd.tile([128, 1], F32, tag="rs9")
                    nc.vector.tensor_reduce(
                        rs9[:].rearrange("a b -> a b ()"), e9[:], AX.X, AL.add)
                    z_ps = zps.tile([1, 1], F32, tag="z_ps")
                    nc.tensor.matmul(z_ps[:], ones128[:], rs9[:], start=True, stop=True)
                    z_sb = rnd.tile([1, 1], F32, tag="z_sb")
                    nc.scalar.copy(z_sb[:], z_ps[:])
                    zb_ps = zps.tile([128, 1], F32, tag="zb_ps")
                    nc.tensor.matmul(zb_ps[:], ones1[:], z_sb[:], start=True, stop=True)
                    rz = rnd.tile([128, 1], F32, tag="rz")
                    nc.vector.reciprocal(rz[:], zb_ps[:])
                    ce9f = rnd.tile([128, 9], F32, tag="ce9f")
                    nc.vector.tensor_scalar_mul(ce9f[:], e9[:], rz[:])
                    ce9b = rnd.tile([128, 9], BF16, tag="ce9b")
                    nc.scalar.copy(ce9b[:], ce9f[:])
                    return ce9f, ce9b

                def scale_xrT(m9):
                    """xrT[p, (q,j,b)] *= m9[p, j] in place (bf16)."""
                    nc.vector.tensor_tensor(
                        _r(xrT, [[xrT.ap[0][0], 128], [9 * BC, 8], [BC, 9], [1, BC]]),
                        _r(xrT, [[xrT.ap[0][0], 128], [9 * BC, 8], [BC, 9], [1, BC]]),
                        _r(m9, [[m9.ap[0][0], 128], [0, 8], [1, 9], [0, BC]]),
                        AL.mult,
                    )

                # ---- round 1 (c uniform; xrT unscaled) ----
                s_ps = s_matmul()
                s_sb = rnd.tile([BC, HL], F32, tag="s_sb")
                nc.scalar.mul(s_sb[:], s_ps[:], 1.0 / 1152.0)
                v_sb = squash(s_sb)
                p_delta_update(v_sb, 0, None)
                # ---- round 2 ----
                ce9f_2, ce9b_2 = softmax_ce9()
                scale_xrT(ce9b_2)
                rce9 = rnd.tile([128, 9], F32, tag="rce9")
                nc.vector.reciprocal(rce9[:], ce9f_2[:])
                s_ps = s_matmul()
                s_sb = rnd.tile([BC, HL], F32, tag="s_sb")
                nc.scalar.copy(s_sb[:], s_ps[:])
                v_sb = squash(s_sb)
                p_delta_update(v_sb, 1, rce9)
                # ---- round 3 (b update dead) ----
                ce9f_3, _unused = softmax_ce9()
                ratio9f = rnd.tile([128, 9], F32, tag="ratio9f")
                nc.vector.tensor_tensor(ratio9f[:], ce9f_3[:], rce9[:], AL.mult)
                ratio9b = rnd.tile([128, 9], BF16, tag="ratio9b")
                nc.scalar.copy(ratio9b[:], ratio9f[:])
                scale_xrT(ratio9b)
                s_ps = s_matmul()
                s_sb = rnd.tile([BC, HL], F32, tag="s_sb")
                nc.scalar.copy(s_sb[:], s_ps[:])
                v_sb = squash(s_sb)
                nc.sync.dma_start(vout[:], v_sb[:])

    return nc


_NC_CACHE = None


def _get_nc():
    global _NC_CACHE
    if _NC_CACHE is None:
        nc = build_nc()
        split_waits(nc)
        _NC_CACHE = nc
    return _NC_CACHE


def prepare_inputs(x, conv1_w, conv1_b, pc_w, pc_b, W):
    x = np.asarray(x, np.float32)
    xs = np.zeros((B, 800), np.float32)
    xs[:, :784] = x.reshape(B, 784)
    w1t = np.ascontiguousarray(np.asarray(conv1_w, np.float32).reshape(256, 81).T)
    b1 = np.ascontiguousarray(np.asarray(conv1_b, np.float32))
    pcwt = np.ascontiguousarray(
        np.asarray(pc_w, np.float32).reshape(256, 256, 81).transpose(2, 1, 0))
    pcb = np.ascontiguousarray(np.asarray(pc_b, np.float32).reshape(256))
    w2n = np.ascontiguousarray(
        np.asarray(W, np.float32).transpose(3, 0, 1, 2).reshape(NS, HL))
    w2sb_h = np.ascontiguousarray(
        w2n.reshape(NT, 128, HL).transpose(1, 0, 2).reshape(128, NT * HL)
    ).astype(NPBF16)
    w2nt = np.ascontiguousarray(w2n.T)
    w2nta_h = np.ascontiguousarray(w2nt[:128]).astype(NPBF16)
    w2ntb_h = np.ascontiguousarray(w2nt[128:]).astype(NPBF16)
    eye64 = np.eye(BC, dtype=np.float32)
    in_maps = []
    for c in range(NCORES):
        in_maps.append({
            "xs": np.ascontiguousarray(xs[c * BC:(c + 1) * BC]),
            "w1t": w1t, "b1": b1, "pcwt": pcwt, "pcb": pcb,
            "w2sb_h": w2sb_h, "w2nta_h": w2nta_h, "w2ntb_h": w2ntb_h,
            "eye64": eye64,
        })
    return in_maps


def kernel(x, conv1_w, conv1_b, pc_w, pc_b, W, _trace=False, _trace_kwargs=None):
    nc = _get_nc()
    in_maps = prepare_inputs(x, conv1_w, conv1_b, pc_w, pc_b, W)
    res = run_bass_kernel_spmd(
        nc, in_maps, list(range(NCORES)),
        trace=_trace, **(_trace_kwargs or {}),
    )
    v = np.concatenate([np.asarray(res.results[c]["vout"]) for c in range(NCORES)], 0)
    out = v.reshape(B, 1, 1, 10, 16).astype(np.float32)
    if _trace:
        return out, res
    return out


# revision 65
# speedup vs baseline: 1.3862x; 1.1609x over previous
"""CapsNet forward kernel for Trainium2, 8-core data-parallel.

Strategy (per spec sharding_hint): batch (512) split across 8 cores (64 each);
all params replicated. Routing logits b are a batch-mean -> AllGather of
per-core partial deltas (1152 floats) per routing round (rounds 1,2 only;
round 3's b update is dead in the reference).

Math restructuring (keeps exact semantics, avoids materializing u):
  r := s*1152 + n  (s=caps idx, n=(c32,oy,ox))  == co*36 + pix  with co=s*32+c32
  xr2[b, r]   = primary-caps output (relu), flattened
  W2n[r, hl]  = W.transpose(3,0,1,2).reshape(9216,160)
  s[b,hl]  = sum_r c[n(r)] * W2n[r,hl] * xr2[b,r]        (matmul, K=9216)
  v        = squash_dim1(s)
  G[r,hl]  = sum_b xr2[b,r] * v[b,hl]                    (matmul, K=64/core)
  delta[n] = 1/(B*160) * sum_s sum_hl W2n[r,hl]*G[r,hl]  (DVE TT-reduce)
Convs are PE matmuls: conv1 via in-SBUF "wide patch" im2col (K=81),
primary-caps conv via 81 shifted-window matmuls accumulated in PSUM (K=256).
All big matmuls run as float32r (full-rate fp32 PE mode).
"""

import numpy as np

import concourse.bass as bass
import concourse.mybir as mybir
import concourse.tile as tile
from concourse.ap import AP
from concourse.bass_utils import run_bass_kernel_spmd

F32 = mybir.dt.float32
F32R = mybir.dt.float32r
BF16 = mybir.dt.bfloat16
NPBF16 = mybir.dt.np(mybir.dt.bfloat16)
AL = mybir.AluOpType
AF = mybir.ActivationFunctionType
AX = mybir.AxisListType

NCORES = 8
B = 512
BC = B // NCORES           # 64 images per core
MAX_WAITS = 1              # walrus on this path allows 1 sync wait per inst
HL = 160                   # 10 classes x 16 pose
NS = 9216                  # 1152 caps x 8
NT = NS // 128             # 72 K-tiles
GROUPS = [(0, 14), (14, 14), (28, 14), (42, 14), (56, 8)]  # conv2 image groups
ROUTE_SCALE = 1.0 / (B * HL)


def _r(t, dims):
    """Raw AP on tile/ap t with explicit [step, count] dims (elements)."""
    return AP(t.tensor, t.offset, dims)


def split_waits(nc, max_waits=MAX_WAITS):
    """This walrus build rejects >max_waits sync waits per instruction; move
    excess waits onto same-engine NoOps inserted immediately before."""
    for f in nc.m.functions:
        for blk in f.blocks:
            out = []
            for ins in blk.instructions:
                si = ins.sync_info
                if si is not None and si.on_wait and len(si.on_wait) > max_waits:
                    waits = list(si.on_wait)
                    k = 0
                    while len(waits) > max_waits:
                        chunk, waits = waits[:max_waits], waits[max_waits:]
                        nop = mybir.InstNoOp(name=f"{ins.name}-ws{k}", ins=[], outs=[])
                        nop.engine = ins.engine
                        nop.sync_info = mybir.SyncInfo(on_wait=chunk, on_update=[])
                        out.append(nop)
                        k += 1
                    ins.sync_info = mybir.SyncInfo(
                        on_wait=waits, on_update=list(si.on_update or []))
                out.append(ins)
            blk.instructions = out


def build_nc(stub_collective=False):
    nc = bass.Bass(num_devices=1 if stub_collective else NCORES)

    xs = nc.dram_tensor("xs", [BC, 800], BF16, kind="ExternalInput")
    w1t = nc.dram_tensor("w1t", [81, 256], BF16, kind="ExternalInput")
    b1 = nc.dram_tensor("b1", [256], F32, kind="ExternalInput")
    pcwt = nc.dram_tensor("pcwt", [81, 256, 256], F32R, kind="ExternalInput")
    pcb = nc.dram_tensor("pcb", [256], F32, kind="ExternalInput")
    w2sb_h = nc.dram_tensor("w2sb_h", [128, NT * HL], BF16, kind="ExternalInput")
    w2nta_h = nc.dram_tensor("w2nta_h", [128, NS], BF16, kind="ExternalInput")
    w2ntb_h = nc.dram_tensor("w2ntb_h", [32, NS], BF16, kind="ExternalInput")
    eye64 = nc.dram_tensor("eye64", [BC, BC], F32R, kind="ExternalInput")
    vout = nc.dram_tensor("vout", [BC, HL], F32R, kind="ExternalOutput")

    pc_rd = nc.dram_tensor("pc_rd", [NS, BC], BF16)    # [r, b]

    with tile.TileContext(nc) as tc:
        with (
            tc.tile_pool(name="pers", bufs=1) as pers,
            tc.tile_pool(name="dram", bufs=1, space="DRAM") as dpool,
        ):
            w1t_sb = pers.tile([81, 256], BF16)
            nc.sync.dma_start(w1t_sb[:], w1t[:])
            b1_sb = pers.tile([128, 2], F32)
            nc.sync.dma_start(b1_sb[:], _r(b1[:], [[1, 128], [128, 2]]))
            pcb_sb = pers.tile([128, 2], F32)
            nc.sync.dma_start(pcb_sb[:], _r(pcb[:], [[1, 128], [128, 2]]))
            ones128 = pers.tile([128, 1], F32)
            nc.gpsimd.memset(ones128[:], 1.0)
            ones1 = pers.tile([1, 128], F32)
            nc.gpsimd.memset(ones1[:], 1.0)
            b9 = pers.tile([128, 9], F32)
            eye_sb = pers.tile([BC, BC], F32R)
            nc.sync.dma_start(eye_sb[:], eye64[:])

            # ---------------- conv phase ----------------
            with (
                tc.tile_pool(name="convsb", bufs=1) as csb,
                tc.tile_pool(name="pwp", bufs=6) as pwp,
                tc.tile_pool(name="ps1p", bufs=2, space="PSUM") as ps1p,
                tc.tile_pool(name="ps2p", bufs=1, space="PSUM") as ps2p,
            ):
                acc0 = csb.tile([128, BC * 36], F32)
                acc1 = csb.tile([128, BC * 36], F32)
                accs = [acc0, acc1]
                for ci_blk in range(2):
                    h1 = csb.tile([128, BC * 400], F32R, tag="h1")
                    hp = h1.ap[0][0]
                    for i in range(BC):
                        pw = pwp.tile([81, 560], BF16, tag="pw")
                        nc.sync.dma_start(
                            pw[:],
                            AP(xs[:].tensor, i * 800, [[28, 9], [1, 9], [1, 560]]),
                        )
                        ps1 = ps1p.tile([128, 400], F32, tag="ps1")
                        rhs = _r(pw, [[pw.ap[0][0], 81], [28, 20], [1, 20]])
                        out4 = _r(ps1, [[ps1.ap[0][0], 128], [20, 20], [1, 20]])
                        nc.tensor.matmul(
                            out4,
                            w1t_sb[:, ci_blk * 128:(ci_blk + 1) * 128],
                            rhs,
                            start=True, stop=True,
                        )
                        nc.scalar.activation(
                            h1[:, i * 400:(i + 1) * 400], ps1[:], AF.Relu,
                            bias=b1_sb[:, ci_blk:ci_blk + 1],
                        )
                    for co_blk in range(2):
                        # weights in 3 chunks of 27 taps: loads pipeline with
                        # the kk-outer consumption instead of stalling a chain
                        w2ck = [csb.tile([128, 27 * 128], F32R, tag=f"w2ck{c}",
                                         name=f"w2ck{c}") for c in range(3)]
                        for c in range(3):
                            nc.sync.dma_start(
                                w2ck[c][:],
                                AP(pcwt[:].tensor,
                                   (c * 27) * 256 * 256
                                   + ci_blk * 128 * 256 + co_blk * 128,
                                   [[256, 128], [256 * 256, 27], [1, 128]]),
                            )
                        # kk outer: each stationary slice feeds all 5 image
                        # groups (5 PSUM banks accumulate concurrently)
                        ps2s = [ps2p.tile([128, 504], F32, tag=f"ps2_{gi}",
                                          name=f"ps2_{gi}")
                                for gi in range(len(GROUPS))]
                        for kk in range(81):
                            ky, kx = divmod(kk, 9)
                            wsl = w2ck[kk // 27][:, (kk % 27) * 128:
                                                 (kk % 27 + 1) * 128]
                            for gi, (g0, nb) in enumerate(GROUPS):
                                rhs = AP(h1.tensor,
                                         h1.offset + g0 * 400 + ky * 20 + kx,
                                         [[hp, 128], [400, nb], [40, 6], [2, 6]])
                                out4 = _r(ps2s[gi],
                                          [[ps2s[gi].ap[0][0], 128], [36, nb],
                                           [6, 6], [1, 6]])
                                nc.tensor.matmul(
                                    out4, wsl, rhs,
                                    start=(kk == 0), stop=(kk == 80),
                                )
                        for gi, (g0, nb) in enumerate(GROUPS):
                            dst = accs[co_blk][:, g0 * 36:(g0 + nb) * 36]
                            if ci_blk == 0:
                                nc.scalar.copy(dst, ps2s[gi][:, :nb * 36])
                            else:
                                nc.vector.tensor_tensor(
                                    dst, dst, ps2s[gi][:, :nb * 36], AL.add)
                # bias + relu -> pc2 (pix-major, bf16) -> pc_rd[r, b] in DRAM
                for co_blk in range(2):
                    pc2 = csb.tile([128, BC * 36], BF16, tag="pc2")
                    p2 = pc2.ap[0][0]
                    nc.scalar.activation(
                        _r(pc2, [[p2, 128], [1, BC], [BC, 36]]),
                        _r(accs[co_blk], [[accs[co_blk].ap[0][0], 128], [36, BC], [1, 36]]),
                        AF.Relu,
                        bias=pcb_sb[:, co_blk:co_blk + 1],
                    )
                    nc.sync.dma_start(
                        AP(pc_rd[:].tensor, co_blk * 128 * 36 * BC,
                           [[36 * BC, 128], [BC, 36], [1, BC]]),
                        _r(pc2, [[p2, 128], [BC, 36], [1, BC]]),
                    )

            # ---------------- routing phase ----------------
            with (
                tc.tile_pool(name="rsb", bufs=1) as rsb,
                tc.tile_pool(name="rnd", bufs=2) as rnd,
                tc.tile_pool(name="sps", bufs=1, space="PSUM") as sps,
                tc.tile_pool(name="gps", bufs=4, space="PSUM") as gps,
                tc.tile_pool(name="zps", bufs=1, space="PSUM") as zps,
            ):
                # xrT first so s_matmul can start while weights stream in;
                # w2sb split in two tiles so the first half unblocks early
                xrT = rsb.tile([128, NT * BC], BF16)
                nc.sync.dma_start(
                    xrT[:],
                    AP(pc_rd[:].tensor, 0, [[BC, 128], [128 * BC, NT], [1, BC]]),
                )
                w2sb = [rsb.tile([128, 36 * HL], BF16, name=f"w2sb{h}")
                        for h in range(2)]
                for h in range(2):
                    nc.sync.dma_start(
                        w2sb[h][:],
                        AP(w2sb_h[:].tensor, h * 36 * HL,
                           [[NT * HL, 128], [1, 36 * HL]]),
                    )
                w2nt_a = rsb.tile([128, NT * 128], BF16)
                nc.sync.dma_start(w2nt_a[:], w2nta_h[:])
                w2nt_b = rsb.tile([32, NT * 128], BF16)
                nc.sync.dma_start(w2nt_b[:], w2ntb_h[:])
                p_all = rsb.tile([128, NT * BC], BF16)
                prod = rsb.tile([128, (NT // 2) * BC], F32)

                def s_matmul():
                    s_ps = sps.tile([BC, HL], F32, tag="s_ps")
                    for t in range(NT):
                        nc.tensor.matmul(
                            s_ps[:],
                            xrT[:, t * BC:(t + 1) * BC],
                            w2sb[t // 36][:, (t % 36) * HL:(t % 36 + 1) * HL],
                            start=(t == 0), stop=(t == NT - 1),
                        )
                    return s_ps

                def squash(s_sb):
                    sq = rnd.tile([BC, HL], F32, tag="sq")
                    nc.scalar.square(sq[:], s_sb[:])
                    n2 = rnd.tile([BC, 16], F32, tag="n2")
                    nc.vector.tensor_reduce(
                        n2[:].rearrange("a b -> a b ()"),
                        _r(sq, [[sq.ap[0][0], BC], [1, 16], [16, 10]]),
                        AX.X, AL.add,
                    )
                    rt = rnd.tile([BC, 16], F32, tag="rt")
                    nc.scalar.sqrt(rt[:], n2[:])
                    n2p1 = rnd.tile([BC, 16], F32, tag="n2p1")
                    nc.vector.tensor_scalar_add(n2p1[:], n2[:], 1.0)
                    rcp = rnd.tile([BC, 16], F32, tag="rcp")
                    nc.vector.reciprocal(rcp[:], n2p1[:])
                    f = rnd.tile([BC, 16], F32, tag="f")
                    nc.vector.tensor_tensor(f[:], rt[:], rcp[:], AL.mult)
                    v_sb = rnd.tile([BC, HL], F32R, tag="v_sb")
                    nc.vector.tensor_tensor(
                        _r(v_sb, [[v_sb.ap[0][0], BC], [16, 10], [1, 16]]),
                        _r(s_sb, [[s_sb.ap[0][0], BC], [16, 10], [1, 16]]),
                        _r(f, [[f.ap[0][0], BC], [0, 10], [1, 16]]),
                        AL.mult,
                    )
                    return v_sb

                def p_delta_update(v_sb, rnd_idx, rce9):
                    """delta via P[r,b] = sum_hl W2n[r,hl] v[b,hl] (PE), then
                    D[r] = sum_b xrT[r,b]*P[r,b] (DVE). If xrT is c-scaled,
                    divide delta9 by ce9 (rce9 ap) to undo."""
                    vt_ps = gps.tile([128, BC], F32R, tag="vt_ps", bufs=1)
                    nc.tensor.transpose(vt_ps[:], v_sb[:, 0:128], eye_sb[:])
                    vt_a = rnd.tile([128, BC], BF16, tag="vt_a")
                    nc.scalar.copy(vt_a[:], vt_ps[:])
                    vtb_ps = gps.tile([32, BC], F32R, tag="vtb_ps", bufs=1)
                    nc.tensor.transpose(vtb_ps[:], v_sb[:, 128:160], eye_sb[:])
                    vt_b = rnd.tile([32, BC], BF16, tag="vt_b")
                    nc.scalar.copy(vt_b[:], vtb_ps[:])
                    # 4 K-tiles per PSUM bank -> one psum->bf16 copy per 4
                    for g in range(NT // 4):
                        pps = gps.tile([128, 4 * BC], F32, tag="p_ps", bufs=3)
                        for q in range(4):
                            t = g * 4 + q
                            reg = pps[:, q * BC:(q + 1) * BC]
                            nc.tensor.matmul(
                                reg,
                                w2nt_a[:, t * 128:(t + 1) * 128],
                                vt_a[:],
                                start=True, stop=False,
                            )
                            nc.tensor.matmul(
                                reg,
                                w2nt_b[:, t * 128:(t + 1) * 128],
                                vt_b[:],
                                start=False, stop=True,
                            )
                        nc.scalar.copy(
                            p_all[:, g * 4 * BC:(g + 1) * 4 * BC], pps[:])
                    D = rnd.tile([128, NT], F32, tag="D")
                    half = (NT // 2) * BC
                    for hx in range(2):
                        nc.vector.tensor_tensor(
                            prod[:],
                            xrT[:, hx * half:(hx + 1) * half],
                            p_all[:, hx * half:(hx + 1) * half],
                            AL.mult,
                        )
                        nc.vector.tensor_reduce(
                            D[:, hx * (NT // 2):(hx + 1) * (NT // 2)]
                            .rearrange("a b -> a b ()"),
                            _r(prod, [[prod.ap[0][0], 128], [BC, NT // 2], [1, BC]]),
                            AX.X, AL.add,
                        )
                    delta9 = rnd.tile([128, 9], F32, tag="delta9")
                    nc.vector.tensor_reduce(
                        delta9[:].rearrange("a b -> a b ()"),
                        _r(D, [[D.ap[0][0], 128], [1, 9], [9, 8]]),
                        AX.X, AL.add,
                    )
                    if rce9 is not None:
                        nc.vector.tensor_tensor(delta9[:], delta9[:], rce9[:], AL.mult)
                    cin = dpool.tile([128, 9], F32, name=f"cin{rnd_idx}")
                    cout = dpool.tile([NCORES * 128, 9], F32, name=f"cout{rnd_idx}",
                                      addr_space=("Local" if stub_collective else "Shared"))
                    nc.sync.dma_start(cin[:], delta9[:])
                    if stub_collective:
                        for cc in range(NCORES):
                            nc.gpsimd.dma_start(
                                AP(cout.tensor, cout.offset + cc * 1152,
                                   [[9, 128], [1, 9]]),
                                delta9[:],
                            )
                    else:
                        nc.gpsimd.collective_compute(
                            "AllGather", AL.bypass,
                            replica_groups=[list(range(NCORES))],
                            ins=[cin.opt()], outs=[cout.opt()],
                        )
                    agg = rnd.tile([128, 8 * 9], F32, tag="agg")
                    nc.sync.dma_start(
                        agg[:],
                        AP(cout.tensor, cout.offset, [[9, 128], [1, 9], [128 * 9, 8]]),
                    )
                    dsum = rnd.tile([128, 9], F32, tag="dsum")
                    nc.vector.tensor_reduce(
                        dsum[:].rearrange("a b -> a b ()"),
                        _r(agg, [[agg.ap[0][0], 128], [1, 9], [9, 8]]),
                        AX.X, AL.add,
                    )
                    if rnd_idx == 0:
                        nc.scalar.mul(b9[:], dsum[:], ROUTE_SCALE)
                    else:
                        sc = rnd.tile([128, 9], F32, tag="sc")
                        nc.scalar.mul(sc[:], dsum[:], ROUTE_SCALE)
                        nc.vector.tensor_tensor(b9[:], b9[:], sc[:], AL.add)

                def softmax_ce9():
                    """ce9[p,j] = softmax(b9)[n=j*128+p]: (f32, bf16) pair."""
                    e9 = rnd.tile([128, 9], F32, tag="e9")
                    nc.scalar.activation(e9[:], b9[:], AF.Exp)
                    rs9 = rnd.tile([128, 1], F32, tag="rs9")
                    nc.vector.tensor_reduce(
                        rs9[:].rearrange("a b -> a b ()"), e9[:], AX.X, AL.add)
                    z_ps = zps.tile([1, 1], F32, tag="z_ps")
                    nc.tensor.matmul(z_ps[:], ones128[:], rs9[:], start=True, stop=True)
                    z_sb = rnd.tile([1, 1], F32, tag="z_sb")
                    nc.scalar.copy(z_sb[:], z_ps[:])
                    zb_ps = zps.tile([128, 1], F32, tag="zb_ps")
                    nc.tensor.matmul(zb_ps[:], ones1[:], z_sb[:], start=True, stop=True)
                    rz = rnd.tile([128, 1], F32, tag="rz")
                    nc.vector.reciprocal(rz[:], zb_ps[:])
                    ce9f = rnd.tile([128, 9], F32, tag="ce9f")
                    nc.vector.tensor_scalar_mul(ce9f[:], e9[:], rz[:])
                    ce9b = rnd.tile([128, 9], BF16, tag="ce9b")
                    nc.scalar.copy(ce9b[:], ce9f[:])
                    return ce9f, ce9b

                def scale_xrT(m9):
                    """xrT[p, (q,j,b)] *= m9[p, j] in place (bf16)."""
                    nc.vector.tensor_tensor(
                        _r(xrT, [[xrT.ap[0][0], 128], [9 * BC, 8], [BC, 9], [1, BC]]),
                        _r(xrT, [[xrT.ap[0][0], 128], [9 * BC, 8], [BC, 9], [1, BC]]),
                        _r(m9, [[m9.ap[0][0], 128], [0, 8], [1, 9], [0, BC]]),
                        AL.mult,
                    )

                # ---- round 1 (c uniform; xrT unscaled) ----
                s_ps = s_matmul()
                s_sb = rnd.tile([BC, HL], F32, tag="s_sb")
                nc.scalar.mul(s_sb[:], s_ps[:], 1.0 / 1152.0)
                v_sb = squash(s_sb)
                p_delta_update(v_sb, 0, None)
                # ---- round 2 ----
                ce9f_2, ce9b_2 = softmax_ce9()
                scale_xrT(ce9b_2)
                rce9 = rnd.tile([128, 9], F32, tag="rce9")
                nc.vector.reciprocal(rce9[:], ce9f_2[:])
                s_ps = s_matmul()
                s_sb = rnd.tile([BC, HL], F32, tag="s_sb")
                nc.scalar.copy(s_sb[:], s_ps[:])
                v_sb = squash(s_sb)
                p_delta_update(v_sb, 1, rce9)
                # ---- round 3 (b update dead) ----
                ce9f_3, _unused = softmax_ce9()
                ratio9f = rnd.tile([128, 9], F32, tag="ratio9f")
                nc.vector.tensor_tensor(ratio9f[:], ce9f_3[:], rce9[:], AL.mult)
                ratio9b = rnd.tile([128, 9], BF16, tag="ratio9b")
                nc.scalar.copy(ratio9b[:], ratio9f[:])
                scale_xrT(ratio9b)
                s_ps = s_matmul()
                s_sb = rnd.tile([BC, HL], F32, tag="s_sb")
                nc.scalar.copy(s_sb[:], s_ps[:])
                v_sb = squash(s_sb)
                nc.sync.dma_start(vout[:], v_sb[:])

    return nc


_NC_CACHE = None


def _get_nc():
    global _NC_CACHE
    if _NC_CACHE is None:
        nc = build_nc()
        split_waits(nc)
        _NC_CACHE = nc
    return _NC_CACHE


def prepare_inputs(x, conv1_w, conv1_b, pc_w, pc_b, W):
    x = np.asarray(x, np.float32)
    xs = np.zeros((B, 800), np.float32)
    xs[:, :784] = x.reshape(B, 784)
    xs = xs.astype(NPBF16)
    w1t = np.ascontiguousarray(
        np.asarray(conv1_w, np.float32).reshape(256, 81).T).astype(NPBF16)
    b1 = np.ascontiguousarray(np.asarray(conv1_b, np.float32))
    pcwt = np.ascontiguousarray(
        np.asarray(pc_w, np.float32).reshape(256, 256, 81).transpose(2, 1, 0))
    pcb = np.ascontiguousarray(np.asarray(pc_b, np.float32).reshape(256))
    w2n = np.ascontiguousarray(
        np.asarray(W, np.float32).transpose(3, 0, 1, 2).reshape(NS, HL))
    w2sb_h = np.ascontiguousarray(
        w2n.reshape(NT, 128, HL).transpose(1, 0, 2).reshape(128, NT * HL)
    ).astype(NPBF16)
    w2nt = np.ascontiguousarray(w2n.T)
    w2nta_h = np.ascontiguousarray(w2nt[:128]).astype(NPBF16)
    w2ntb_h = np.ascontiguousarray(w2nt[128:]).astype(NPBF16)
    eye64 = np.eye(BC, dtype=np.float32)
    in_maps = []
    for c in range(NCORES):
        in_maps.append({
            "xs": np.ascontiguousarray(xs[c * BC:(c + 1) * BC]),
            "w1t": w1t, "b1": b1, "pcwt": pcwt, "pcb": pcb,
            "w2sb_h": w2sb_h, "w2nta_h": w2nta_h, "w2ntb_h": w2ntb_h,
            "eye64": eye64,
        })
    return in_maps


def kernel(x, conv1_w, conv1_b, pc_w, pc_b, W, _trace=False, _trace_kwargs=None):
    nc = _get_nc()
    in_maps = prepare_inputs(x, conv1_w, conv1_b, pc_w, pc_b, W)
    res = run_bass_kernel_spmd(
        nc, in_maps, list(range(NCORES)),
        trace=_trace, **(_trace_kwargs or {}),
    )
    v = np.concatenate([np.asarray(res.results[c]["vout"]) for c in range(NCORES)], 0)
    out = v.reshape(B, 1, 1, 10, 16).astype(np.float32)
    if _trace:
        return out, res
    return out


# revision 81
# speedup vs baseline: 1.4254x; 1.0283x over previous
"""CapsNet forward kernel for Trainium2, 8-core data-parallel.

Strategy (per spec sharding_hint): batch (512) split across 8 cores (64 each);
all params replicated. Routing logits b are a batch-mean -> AllGather of
per-core partial deltas (1152 floats) per routing round (rounds 1,2 only;
round 3's b update is dead in the reference).

Math restructuring (keeps exact semantics, avoids materializing u):
  r := s*1152 + n  (s=caps idx, n=(c32,oy,ox))  == co*36 + pix  with co=s*32+c32
  xr2[b, r]   = primary-caps output (relu), flattened
  W2n[r, hl]  = W.transpose(3,0,1,2).reshape(9216,160)
  s[b,hl]  = sum_r c[n(r)] * W2n[r,hl] * xr2[b,r]        (matmul, K=9216)
  v        = squash_dim1(s)
  G[r,hl]  = sum_b xr2[b,r] * v[b,hl]                    (matmul, K=64/core)
  delta[n] = 1/(B*160) * sum_s sum_hl W2n[r,hl]*G[r,hl]  (DVE TT-reduce)
Convs are PE matmuls: conv1 via in-SBUF "wide patch" im2col (K=81),
primary-caps conv via 81 shifted-window matmuls accumulated in PSUM (K=256).
All big matmuls run as float32r (full-rate fp32 PE mode).
"""

import numpy as np

import concourse.bass as bass
import concourse.mybir as mybir
import concourse.tile as tile
from concourse.ap import AP
from concourse.bass_utils import run_bass_kernel_spmd

F32 = mybir.dt.float32
F32R = mybir.dt.float32r
BF16 = mybir.dt.bfloat16
NPBF16 = mybir.dt.np(mybir.dt.bfloat16)
AL = mybir.AluOpType
AF = mybir.ActivationFunctionType
AX = mybir.AxisListType

NCORES = 8
B = 512
BC = B // NCORES           # 64 images per core
MAX_WAITS = 1              # walrus on this path allows 1 sync wait per inst
HL = 160                   # 10 classes x 16 pose
NS = 9216                  # 1152 caps x 8
NT = NS // 128             # 72 K-tiles
GROUPS = [(0, 14), (14, 14), (28, 14), (42, 14), (56, 8)]  # conv2 image groups
ROUTE_SCALE = 1.0 / (B * HL)


def _r(t, dims):
    """Raw AP on tile/ap t with explicit [step, count] dims (elements)."""
    return AP(t.tensor, t.offset, dims)


def split_waits(nc, max_waits=MAX_WAITS):
    """This walrus build rejects >max_waits sync waits per instruction; move
    excess waits onto same-engine NoOps inserted immediately before."""
    for f in nc.m.functions:
        for blk in f.blocks:
            out = []
            for ins in blk.instructions:
                si = ins.sync_info
                if si is not None and si.on_wait and len(si.on_wait) > max_waits:
                    waits = list(si.on_wait)
                    k = 0
                    while len(waits) > max_waits:
                        chunk, waits = waits[:max_waits], waits[max_waits:]
                        nop = mybir.InstNoOp(name=f"{ins.name}-ws{k}", ins=[], outs=[])
                        nop.engine = ins.engine
                        nop.sync_info = mybir.SyncInfo(on_wait=chunk, on_update=[])
                        out.append(nop)
                        k += 1
                    ins.sync_info = mybir.SyncInfo(
                        on_wait=waits, on_update=list(si.on_update or []))
                out.append(ins)
            blk.instructions = out


def build_nc(stub_collective=False):
    nc = bass.Bass(num_devices=1 if stub_collective else NCORES)

    xs = nc.dram_tensor("xs", [BC, 800], BF16, kind="ExternalInput")
    w1t = nc.dram_tensor("w1t", [81, 256], BF16, kind="ExternalInput")
    b1 = nc.dram_tensor("b1", [256], F32, kind="ExternalInput")
    pcwt = nc.dram_tensor("pcwt", [81, 256, 256], F32R, kind="ExternalInput")
    pcb = nc.dram_tensor("pcb", [256], F32, kind="ExternalInput")
    w2sb_h = nc.dram_tensor("w2sb_h", [128, NT * HL], BF16, kind="ExternalInput")
    w2nta_h = nc.dram_tensor("w2nta_h", [128, NS], BF16, kind="ExternalInput")
    w2ntb_h = nc.dram_tensor("w2ntb_h", [32, NS], BF16, kind="ExternalInput")
    eye64 = nc.dram_tensor("eye64", [BC, BC], F32R, kind="ExternalInput")
    selm = nc.dram_tensor("selm", [72, 9], F32, kind="ExternalInput")
    vout = nc.dram_tensor("vout", [BC, HL], F32R, kind="ExternalOutput")

    pc_rd = nc.dram_tensor("pc_rd", [NS, BC], BF16)    # [r, b]

    with tile.TileContext(nc) as tc:
        with (
            tc.tile_pool(name="pers", bufs=1) as pers,
            tc.tile_pool(name="dram", bufs=1, space="DRAM") as dpool,
        ):
            w1t_sb = pers.tile([81, 256], BF16)
            nc.sync.dma_start(w1t_sb[:], w1t[:])
            b1_sb = pers.tile([128, 2], F32)
            nc.sync.dma_start(b1_sb[:], _r(b1[:], [[1, 128], [128, 2]]))
            pcb_sb = pers.tile([128, 2], F32)
            nc.sync.dma_start(pcb_sb[:], _r(pcb[:], [[1, 128], [128, 2]]))
            ones128 = pers.tile([128, 1], F32)
            nc.gpsimd.memset(ones128[:], 1.0)
            ones1 = pers.tile([1, 128], F32)
            nc.gpsimd.memset(ones1[:], 1.0)
            b9 = pers.tile([128, 9], F32)
            eye_sb = pers.tile([BC, BC], F32R)
            nc.sync.dma_start(eye_sb[:], eye64[:])
            sel_sb = pers.tile([72, 9], F32)
            nc.sync.dma_start(sel_sb[:], selm[:])
            # routing weights live in pers so they can prefetch during conv
            w2nt_a = pers.tile([128, NT * 128], BF16)
            xrT = pers.tile([128, NT * BC], BF16)

            # ---------------- conv phase ----------------
            with (
                tc.tile_pool(name="convsb", bufs=1) as csb,
                tc.tile_pool(name="pwp", bufs=4) as pwp,
                tc.tile_pool(name="ps1p", bufs=2, space="PSUM") as ps1p,
                tc.tile_pool(name="ps2p", bufs=1, space="PSUM") as ps2p,
            ):
                acc0 = csb.tile([128, BC * 36], F32)
                acc1 = csb.tile([128, BC * 36], F32)
                accs = [acc0, acc1]

                def finalize(co_blk):
                    # bias + relu -> pc2 (pix-major, bf16) -> pc_rd[r, b]
                    pc2 = csb.tile([128, BC * 36], BF16, tag="pc2",
                                   name=f"pc2_{co_blk}")
                    p2 = pc2.ap[0][0]
                    nc.scalar.activation(
                        _r(pc2, [[p2, 128], [1, BC], [BC, 36]]),
                        _r(accs[co_blk],
                           [[accs[co_blk].ap[0][0], 128], [36, BC], [1, 36]]),
                        AF.Relu,
                        bias=pcb_sb[:, co_blk:co_blk + 1],
                    )
                    nc.sync.dma_start(
                        AP(pc_rd[:].tensor, co_blk * 128 * 36 * BC,
                           [[36 * BC, 128], [BC, 36], [1, BC]]),
                        _r(pc2, [[p2, 128], [BC, 36], [1, BC]]),
                    )
                    # pull this half straight back in r-major for routing
                    nc.sync.dma_start(
                        xrT[:, co_blk * 36 * BC:(co_blk + 1) * 36 * BC],
                        AP(pc_rd[:].tensor, co_blk * 36 * 128 * BC,
                           [[BC, 128], [128 * BC, 36], [1, BC]]),
                    )
                for ci_blk in range(2):
                    h1 = csb.tile([128, BC * 400], F32R, tag="h1")
                    hp = h1.ap[0][0]
                    for i in range(BC):
                        pw = pwp.tile([81, 560], BF16, tag="pw")
                        nc.sync.dma_start(
                            pw[:],
                            AP(xs[:].tensor, i * 800, [[28, 9], [1, 9], [1, 560]]),
                        )
                        ps1 = ps1p.tile([128, 400], F32, tag="ps1")
                        rhs = _r(pw, [[pw.ap[0][0], 81], [28, 20], [1, 20]])
                        out4 = _r(ps1, [[ps1.ap[0][0], 128], [20, 20], [1, 20]])
                        nc.tensor.matmul(
                            out4,
                            w1t_sb[:, ci_blk * 128:(ci_blk + 1) * 128],
                            rhs,
                            start=True, stop=True,
                        )
                        nc.scalar.activation(
                            h1[:, i * 400:(i + 1) * 400], ps1[:], AF.Relu,
                            bias=b1_sb[:, ci_blk:ci_blk + 1],
                        )
                    for co_blk in range(2):
                        # weights in 3 chunks of 27 taps: loads pipeline with
                        # the kk-outer consumption instead of stalling a chain
                        w2ck = [csb.tile([128, 27 * 128], F32R, tag=f"w2ck{c}",
                                         name=f"w2ck{c}") for c in range(3)]
                        for c in range(3):
                            nc.sync.dma_start(
                                w2ck[c][:],
                                AP(pcwt[:].tensor,
                                   (c * 27) * 256 * 256
                                   + ci_blk * 128 * 256 + co_blk * 128,
                                   [[256, 128], [256 * 256, 27], [1, 128]]),
                            )
                        if ci_blk == 1 and co_blk == 1:
                            # co0's accs are final: ship its half of pc_rd
                            # while the last chain computes
                            finalize(0)
                        # kk outer: each stationary slice feeds all 5 image
                        # groups (5 PSUM banks accumulate concurrently)
                        ps2s = [ps2p.tile([128, 504], F32, tag=f"ps2_{gi}",
                                          name=f"ps2_{gi}")
                                for gi in range(len(GROUPS))]
                        for kk in range(81):
                            ky, kx = divmod(kk, 9)
                            wsl = w2ck[kk // 27][:, (kk % 27) * 128:
                                                 (kk % 27 + 1) * 128]
                            for gi, (g0, nb) in enumerate(GROUPS):
                                rhs = AP(h1.tensor,
                                         h1.offset + g0 * 400 + ky * 20 + kx,
                                         [[hp, 128], [400, nb], [40, 6], [2, 6]])
                                out4 = _r(ps2s[gi],
                                          [[ps2s[gi].ap[0][0], 128], [36, nb],
                                           [6, 6], [1, 6]])
                                nc.tensor.matmul(
                                    out4, wsl, rhs,
                                    start=(kk == 0), stop=(kk == 80),
                                )
                        for gi, (g0, nb) in enumerate(GROUPS):
                            dst = accs[co_blk][:, g0 * 36:(g0 + nb) * 36]
                            if ci_blk == 0:
                                nc.scalar.copy(dst, ps2s[gi][:, :nb * 36])
                            else:
                                nc.vector.tensor_tensor(
                                    dst, dst, ps2s[gi][:, :nb * 36], AL.add)
                    if ci_blk == 0:
                        # prefetch routing weights during the second ci pass
                        nc.sync.dma_start(w2nt_a[:], w2nta_h[:])
                finalize(1)

            # ---------------- routing phase ----------------
            with (
                tc.tile_pool(name="rsb", bufs=1) as rsb,
                tc.tile_pool(name="rnd", bufs=2) as rnd,
                tc.tile_pool(name="sps", bufs=1, space="PSUM") as sps,
                tc.tile_pool(name="gps", bufs=4, space="PSUM") as gps,
                tc.tile_pool(name="zps", bufs=1, space="PSUM") as zps,
            ):
                # w2sb split in two tiles so the first half unblocks early
                w2nt_b = rsb.tile([32, NT * 128], BF16)
                nc.sync.dma_start(w2nt_b[:], w2ntb_h[:])
                w2sb = [rsb.tile([128, 36 * HL], BF16, name=f"w2sb{h}")
                        for h in range(2)]
                for h in range(2):
                    nc.sync.dma_start(
                        w2sb[h][:],
                        AP(w2sb_h[:].tensor, h * 36 * HL,
                           [[NT * HL, 128], [1, 36 * HL]]),
                    )
                p_all = rsb.tile([128, NT * BC], BF16)
                prod = rsb.tile([128, (NT // 4) * BC], F32)

                def s_matmul():
                    s_ps = sps.tile([BC, HL], F32, tag="s_ps")
                    for t in range(NT):
                        nc.tensor.matmul(
                            s_ps[:],
                            xrT[:, t * BC:(t + 1) * BC],
                            w2sb[t // 36][:, (t % 36) * HL:(t % 36 + 1) * HL],
                            start=(t == 0), stop=(t == NT - 1),
                        )
                    return s_ps

                def squash(s_sb):
                    sq = rnd.tile([BC, HL], F32, tag="sq")
                    nc.scalar.square(sq[:], s_sb[:])
                    n2 = rnd.tile([BC, 16], F32, tag="n2")
                    nc.vector.tensor_reduce(
                        n2[:].rearrange("a b -> a b ()"),
                        _r(sq, [[sq.ap[0][0], BC], [1, 16], [16, 10]]),
                        AX.X, AL.add,
                    )
                    rt = rnd.tile([BC, 16], F32, tag="rt")
                    nc.scalar.sqrt(rt[:], n2[:])
                    n2p1 = rnd.tile([BC, 16], F32, tag="n2p1")
                    nc.vector.tensor_scalar_add(n2p1[:], n2[:], 1.0)
                    rcp = rnd.tile([BC, 16], F32, tag="rcp")
                    nc.vector.reciprocal(rcp[:], n2p1[:])
                    f = rnd.tile([BC, 16], F32, tag="f")
                    nc.vector.tensor_tensor(f[:], rt[:], rcp[:], AL.mult)
                    v_sb = rnd.tile([BC, HL], F32R, tag="v_sb")
                    nc.vector.tensor_tensor(
                        _r(v_sb, [[v_sb.ap[0][0], BC], [16, 10], [1, 16]]),
                        _r(s_sb, [[s_sb.ap[0][0], BC], [16, 10], [1, 16]]),
                        _r(f, [[f.ap[0][0], BC], [0, 10], [1, 16]]),
                        AL.mult,
                    )
                    return v_sb

                def p_delta_update(v_sb, rnd_idx, rce9):
                    """delta via P[r,b] = sum_hl W2n[r,hl] v[b,hl] (PE), then
                    D[r] = sum_b xrT[r,b]*P[r,b] (DVE). If xrT is c-scaled,
                    divide delta9 by ce9 (rce9 ap) to undo."""
                    vt_ps = gps.tile([128, BC], F32R, tag="vt_ps", bufs=1)
                    nc.tensor.transpose(vt_ps[:], v_sb[:, 0:128], eye_sb[:])
                    vt_a = rnd.tile([128, BC], BF16, tag="vt_a")
                    nc.scalar.copy(vt_a[:], vt_ps[:])
                    vtb_ps = gps.tile([32, BC], F32R, tag="vtb_ps", bufs=1)
                    nc.tensor.transpose(vtb_ps[:], v_sb[:, 128:160], eye_sb[:])
                    vt_b = rnd.tile([32, BC], BF16, tag="vt_b")
                    nc.scalar.copy(vt_b[:], vtb_ps[:])
                    # 4 K-tiles per PSUM bank -> one psum->bf16 copy per 4
                    for g in range(NT // 4):
                        pps = gps.tile([128, 4 * BC], F32, tag="p_ps", bufs=3)
                        for q in range(4):
                            t = g * 4 + q
                            reg = pps[:, q * BC:(q + 1) * BC]
                            nc.tensor.matmul(
                                reg,
                                w2nt_a[:, t * 128:(t + 1) * 128],
                                vt_a[:],
                                start=True, stop=False,
                            )
                            nc.tensor.matmul(
                                reg,
                                w2nt_b[:, t * 128:(t + 1) * 128],
                                vt_b[:],
                                start=False, stop=True,
                            )
                        nc.scalar.copy(
                            p_all[:, g * 4 * BC:(g + 1) * 4 * BC], pps[:])
                    D = rnd.tile([128, NT], F32, tag="D")
                    qn = (NT // 4) * BC
                    for hx in range(4):
                        nc.vector.tensor_tensor(
                            prod[:],
                            xrT[:, hx * qn:(hx + 1) * qn],
                            p_all[:, hx * qn:(hx + 1) * qn],
                            AL.mult,
                        )
                        nc.vector.tensor_reduce(
                            D[:, hx * (NT // 4):(hx + 1) * (NT // 4)]
                            .rearrange("a b -> a b ()"),
                            _r(prod, [[prod.ap[0][0], 128], [BC, NT // 4], [1, BC]]),
                            AX.X, AL.add,
                        )
                    delta9 = rnd.tile([128, 9], F32, tag="delta9")
                    nc.vector.tensor_reduce(
                        delta9[:].rearrange("a b -> a b ()"),
                        _r(D, [[D.ap[0][0], 128], [1, 9], [9, 8]]),
                        AX.X, AL.add,
                    )
                    if rce9 is not None:
                        nc.vector.tensor_tensor(delta9[:], delta9[:], rce9[:], AL.mult)
                    cin = dpool.tile([128, 9], F32, name=f"cin{rnd_idx}")
                    cout = dpool.tile([NCORES * 128, 9], F32, name=f"cout{rnd_idx}",
                                      addr_space=("Local" if stub_collective else "Shared"))
                    nc.sync.dma_start(cin[:], delta9[:])
                    if stub_collective:
                        for cc in range(NCORES):
                            nc.gpsimd.dma_start(
                                AP(cout.tensor, cout.offset + cc * 1152,
                                   [[9, 128], [1, 9]]),
                                delta9[:],
                            )
                    else:
                        nc.gpsimd.collective_compute(
                            "AllGather", AL.bypass,
                            replica_groups=[list(range(NCORES))],
                            ins=[cin.opt()], outs=[cout.opt()],
                        )
                    agg = rnd.tile([128, 8 * 9], F32, tag="agg")
                    nc.sync.dma_start(
                        agg[:],
                        AP(cout.tensor, cout.offset, [[9, 128], [1, 9], [128 * 9, 8]]),
                    )
                    dsum = rnd.tile([128, 9], F32, tag="dsum")
                    nc.vector.tensor_reduce(
                        dsum[:].rearrange("a b -> a b ()"),
                        _r(agg, [[agg.ap[0][0], 128], [1, 9], [9, 8]]),
                        AX.X, AL.add,
                    )
                    if rnd_idx == 0:
                        nc.scalar.mul(b9[:], dsum[:], ROUTE_SCALE)
                    else:
                        sc = rnd.tile([128, 9], F32, tag="sc")
                        nc.scalar.mul(sc[:], dsum[:], ROUTE_SCALE)
                        nc.vector.tensor_tensor(b9[:], b9[:], sc[:], AL.add)

                def softmax_ce9():
                    """ce9[p,j] = softmax(b9)[n=j*128+p]: (f32, bf16) pair."""
                    e9 = rnd.tile([128, 9], F32, tag="e9")
                    nc.scalar.activation(e9[:], b9[:], AF.Exp)
                    rs9 = rnd.tile([128, 1], F32, tag="rs9")
                    nc.vector.tensor_reduce(
                        rs9[:].rearrange("a b -> a b ()"), e9[:], AX.X, AL.add)
                    z_ps = zps.tile([1, 1], F32, tag="z_ps")
                    nc.tensor.matmul(z_ps[:], ones128[:], rs9[:], start=True, stop=True)
                    z_sb = rnd.tile([1, 1], F32, tag="z_sb")
                    nc.scalar.copy(z_sb[:], z_ps[:])
                    zb_ps = zps.tile([128, 1], F32, tag="z_ps", name="zb_ps")
                    nc.tensor.matmul(zb_ps[:], ones1[:], z_sb[:], start=True, stop=True)
                    rz = rnd.tile([128, 1], F32, tag="rz")
                    nc.vector.reciprocal(rz[:], zb_ps[:])
                    ce9f = rnd.tile([128, 9], F32, tag="ce9f")
                    nc.vector.tensor_scalar_mul(ce9f[:], e9[:], rz[:])
                    ce9b = rnd.tile([128, 9], BF16, tag="ce9b")
                    nc.scalar.copy(ce9b[:], ce9f[:])
                    return ce9f, ce9b

                def scale_xrT(m9):
                    """xrT[p, (q,j,b)] *= m9[p, j] in place (bf16).
                    Quartered ascending: s_matmul (ascending t) starts after
                    the first quarter while the rest rescales."""
                    for sub in range(4):
                        v4 = [[xrT.ap[0][0], 128], [9 * BC, 2], [BC, 9], [1, BC]]
                        nc.vector.tensor_tensor(
                            AP(xrT.tensor, xrT.offset + sub * 2 * 9 * BC, v4),
                            AP(xrT.tensor, xrT.offset + sub * 2 * 9 * BC, v4),
                            _r(m9, [[m9.ap[0][0], 128], [0, 2], [1, 9], [0, BC]]),
                            AL.mult,
                        )

                # ---- round 1 (c uniform; xrT unscaled) ----
                s_ps = s_matmul()
                s_sb = rnd.tile([BC, HL], F32, tag="s_sb")
                nc.scalar.mul(s_sb[:], s_ps[:], 1.0 / 1152.0)
                v_sb = squash(s_sb)
                p_delta_update(v_sb, 0, None)
                # ---- round 2 ----
                ce9f_2, ce9b_2 = softmax_ce9()
                scale_xrT(ce9b_2)
                rce9 = rnd.tile([128, 9], F32, tag="rce9")
                nc.vector.reciprocal(rce9[:], ce9f_2[:])
                s_ps = s_matmul()
                s_sb = rnd.tile([BC, HL], F32, tag="s_sb")
                nc.scalar.copy(s_sb[:], s_ps[:])
                v_sb = squash(s_sb)
                p_delta_update(v_sb, 1, rce9)
                # ---- round 3 (b update dead) ----
                ce9f_3, _unused = softmax_ce9()
                ratio9f = rnd.tile([128, 9], F32, tag="ratio9f")
                nc.vector.tensor_tensor(ratio9f[:], ce9f_3[:], rce9[:], AL.mult)
                ratio9b = rnd.tile([128, 9], BF16, tag="ratio9b")
                nc.scalar.copy(ratio9b[:], ratio9f[:])
                scale_xrT(ratio9b)
                s_ps = s_matmul()
                s_sb = rnd.tile([BC, HL], F32, tag="s_sb")
                nc.scalar.copy(s_sb[:], s_ps[:])
                v_sb = squash(s_sb)
                nc.sync.dma_start(vout[:], v_sb[:])

    return nc


_NC_CACHE = None


def _get_nc():
    global _NC_CACHE
    if _NC_CACHE is None:
        nc = build_nc()
        split_waits(nc)
        _NC_CACHE = nc
    return _NC_CACHE


def prepare_inputs(x, conv1_w, conv1_b, pc_w, pc_b, W):
    x = np.asarray(x, np.float32)
    xs = np.zeros((B, 800), np.float32)
    xs[:, :784] = x.reshape(B, 784)
    xs = xs.astype(NPBF16)
    w1t = np.ascontiguousarray(
        np.asarray(conv1_w, np.float32).reshape(256, 81).T).astype(NPBF16)
    b1 = np.ascontiguousarray(np.asarray(conv1_b, np.float32))
    pcwt = np.ascontiguousarray(
        np.asarray(pc_w, np.float32).reshape(256, 256, 81).transpose(2, 1, 0))
    pcb = np.ascontiguousarray(np.asarray(pc_b, np.float32).reshape(256))
    w2n = np.ascontiguousarray(
        np.asarray(W, np.float32).transpose(3, 0, 1, 2).reshape(NS, HL))
    w2sb_h = np.ascontiguousarray(
        w2n.reshape(NT, 128, HL).transpose(1, 0, 2).reshape(128, NT * HL)
    ).astype(NPBF16)
    w2nt = np.ascontiguousarray(w2n.T)
    w2nta_h = np.ascontiguousarray(w2nt[:128]).astype(NPBF16)
    w2ntb_h = np.ascontiguousarray(w2nt[128:]).astype(NPBF16)
    eye64 = np.eye(BC, dtype=np.float32)
    selm = np.zeros((72, 9), np.float32)
    for c in range(8):
        for j in range(9):
            selm[c * 9 + j, j] = 1.0
    in_maps = []
    for c in range(NCORES):
        in_maps.append({
            "xs": np.ascontiguousarray(xs[c * BC:(c + 1) * BC]),
            "w1t": w1t, "b1": b1, "pcwt": pcwt, "pcb": pcb,
            "w2sb_h": w2sb_h, "w2nta_h": w2nta_h, "w2ntb_h": w2ntb_h,
            "eye64": eye64, "selm": selm,
        })
    return in_maps


def kernel(x, conv1_w, conv1_b, pc_w, pc_b, W, _trace=False, _trace_kwargs=None):
    nc = _get_nc()
    in_maps = prepare_inputs(x, conv1_w, conv1_b, pc_w, pc_b, W)
    res = run_bass_kernel_spmd(
        nc, in_maps, list(range(NCORES)),
        trace=_trace, **(_trace_kwargs or {}),
    )
    v = np.concatenate([np.asarray(res.results[c]["vout"]) for c in range(NCORES)], 0)
    out = v.reshape(B, 1, 1, 10, 16).astype(np.float32)
    if _trace:
        return out, res
    return out
